# revision 11
# baseline (speedup 1.0000x reference)
"""DistMult edge scorer on 8 Trainium2 NeuronCores.

score[r, e] = sigmoid(sum_d h_u[src[r,e], d] * W[r, d] * h_v[dst[r,e], d])

Sharding: edges of each relation are sorted by source node on the host and
split into 8 contiguous slices (one per core).  All tables are bf16.

Per core, per relation, per-edge tensors live in [e(partition), d(free)]:
  - v side: per-edge rows fetched with SWDGE dma_gather (bf16 256B rows, one
    descriptor per edge, 4 big calls per relation on rotating queues; pad
    slots use index -1 so their descriptors are skipped).
  - u side: the core's rows are W-prescaled on the host and scattered into
    128-row blocks; edges are packed so chunk t of 128 edges uses rows of
    the single compile-time block blk_t = t*NB//T2.  PE expands per edge:
    ue[e,d] = matmul(lhsT=onehot[row,e], rhs=u_block[row,d]).
  - one-hots are DVE is_equal(ids, iota) in bf16 (ids host-computed u8).
  - ACT evicts the PSUM expansion to bf16; DVE scalar_tensor_tensor fuses
    the v-multiply and the d-reduction (accum_out) straight into an SBUF
    [128, T2] scoreboard; ACT applies sigmoid; host casts/unpermutes.
"""

import numpy as np
import ml_dtypes

BF16 = ml_dtypes.bfloat16

N_DRUG, N_DIS, D = 8000, 18000, 128
N_REL_DIR, E = 3, 200000
N_CORES = 8
EPC = E // N_CORES            # 25000 edges per core per relation

T2 = 224                      # chunks per (relation, core)
EL = T2 * 128                 # edge slots
NI = 2048                     # indices per gather call (16 chunks; the per-
                              # engine desc ring holds ~256 descs, so keep
                              # calls ring-resident: 2048/16 = 128 per ring)
NCALL = EL // NI              # gather calls per relation
GRP = 8                       # chunks per group
NG = T2 // GRP                # groups per relation

_cache = {}
_last = {}


def _blk_of(t, nb):
    return t * nb // T2


def _build_nc(cfg):
    import concourse.bacc as bacc
    import concourse.mybir as mybir
    from concourse.tile import TileContext

    f32 = mybir.dt.float32
    bf16 = mybir.dt.bfloat16
    i16 = mybir.dt.int16
    u8 = mybir.dt.uint8

    nblk_f, nblk_r, _t2 = cfg
    assert _t2 == T2
    nblk = {0: nblk_f, 1: nblk_r}

    nc = bacc.Bacc("TRN2", target_bir_lowering=False, debug=False,
                   num_devices=N_CORES, num_swdge_queues=4)

    t_hd = nc.dram_tensor("hd", (N_DRUG, D), f32, kind="ExternalInput")
    t_hs = nc.dram_tensor("hs", (N_DIS, D), f32, kind="ExternalInput")
    t_u = [nc.dram_tensor(f"u{r}", (nblk[r >= 3] * 128, D), bf16,
                          kind="ExternalInput") for r in range(6)]
    t_iota = nc.dram_tensor("iota", (128, 1), f32, kind="ExternalInput")
    t_ids = [nc.dram_tensor(f"ids{r}", (128, EL), u8,
                            kind="ExternalInput") for r in range(6)]
    t_iv = [nc.dram_tensor(f"iv{r}", (128, EL // 16), i16,
                           kind="ExternalInput") for r in range(6)]
    t_out = [nc.dram_tensor(f"scores{r}", (128, T2), bf16,
                            kind="ExternalOutput") for r in range(6)]

    with TileContext(nc) as tc:
        with tc.tile_pool(name="cst", bufs=1) as cst, \
             tc.tile_pool(name="mp", bufs=2) as mp, \
             tc.tile_pool(name="gvp", bufs=7) as gvp, \
             tc.tile_pool(name="ohp", bufs=3) as ohp, \
             tc.tile_pool(name="evp", bufs=3) as evp, \
             tc.tile_pool(name="pdp", bufs=3) as pdp, \
             tc.tile_pool(name="pue", bufs=3, space="PSUM") as pue:
            iota = cst.tile([128, 1], f32)
            nc.sync.dma_start(iota[:], t_iota[:])

            swdge_calls = 0
            for r in range(6):
                dr = int(r >= 3)
                NB = nblk[dr]
                v_tab = t_hs if dr == 0 else t_hd

                u_sb = mp.tile([128, NB, D], bf16, tag=f"usb{dr}")
                nc.sync.dma_start(
                    u_sb[:], t_u[r][:].rearrange("(b p) d -> p b d", p=128))
                ids = mp.tile([128, EL], u8, tag="ids")
                nc.sync.dma_start(ids[:], t_ids[r][:])
                iv = mp.tile([128, EL // 16], i16, tag="iv")
                nc.sync.dma_start(iv[:], t_iv[r][:])

                gvs = []
                for k in range(NCALL):
                    gv = gvp.tile([128, NI // 128, D], f32, tag="gv")
                    # queue stays congruent with the scheduler's 8-wide
                    # DMASW sem round-robin (each sem is locked to a queue)
                    nc.gpsimd.dma_gather(
                        gv[:], v_tab[:],
                        iv[:, k * (NI // 16):(k + 1) * (NI // 16)],
                        NI, NI, D, elem_step=D, transpose=False,
                        single_packet=False,
                        queue_num=swdge_calls % 4)
                    swdge_calls += 1
                    gvs.append(gv)

                scores = mp.tile([128, T2], f32, tag="scores")
                ohs, ues, evs = {}, {}, {}
                for g in range(NG + 2):
                    if g < NG:
                        oh = ohp.tile([128, GRP, 128], bf16, tag="oh")
                        nc.vector.tensor_scalar(
                            oh[:].rearrange("p a b -> p (a b)"),
                            ids[:, g * 1024:(g + 1) * 1024],
                            iota[:, 0:1], None,
                            op0=mybir.AluOpType.is_equal)
                        ohs[g] = oh
                        ue = pue.tile([128, GRP, D], f32, tag="ue")
                        for c in range(GRP):
                            t = g * GRP + c
                            nc.tensor.matmul(
                                ue[:, c, :],
                                lhsT=oh[:, c, :],
                                rhs=u_sb[:, _blk_of(t, NB), :],
                                start=True, stop=True)
                        ues[g] = ue
                    if 1 <= g <= NG:
                        gp = g - 1
                        ev = evp.tile([128, GRP, D], bf16, tag="ev")
                        nc.scalar.activation(
                            ev[:].rearrange("p a b -> p (a b)"),
                            ues[gp][:].rearrange("p a b -> p (a b)"),
                            mybir.ActivationFunctionType.Copy)
                        evs[gp] = ev
                    if g >= 2:
                        gp = g - 2
                        t0 = gp * GRP
                        ci = (gp % 2) * GRP
                        pd = pdp.tile([128, GRP, D], bf16, tag="pd")
                        nc.vector.tensor_tensor(
                            pd[:].rearrange("p a b -> p (a b)"),
                            evs[gp][:].rearrange("p a b -> p (a b)"),
                            gvs[gp // 2][:, ci:ci + GRP, :].rearrange(
                                "p a b -> p (a b)"),
                            op=mybir.AluOpType.mult)
                        # tree-reduce over d: 128 -> 16 via bf16 2x adds,
                        # then one reduce_sum for the tail
                        s1 = pdp.tile([128, GRP, 64], bf16, tag="s1")
                        nc.vector.tensor_tensor(
                            s1[:], pd[:, :, :64], pd[:, :, 64:],
                            op=mybir.AluOpType.add)
                        s2 = pdp.tile([128, GRP, 32], bf16, tag="s2")
                        nc.vector.tensor_tensor(
                            s2[:], s1[:, :, :32], s1[:, :, 32:],
                            op=mybir.AluOpType.add)
                        s3 = pdp.tile([128, GRP, 16], bf16, tag="s3")
                        nc.vector.tensor_tensor(
                            s3[:], s2[:, :, :16], s2[:, :, 16:],
                            op=mybir.AluOpType.add)
                        nc.vector.reduce_sum(
                            scores[:, t0:t0 + GRP], s3[:],
                            axis=mybir.AxisListType.X)

                sig = mp.tile([128, T2], bf16, tag="sig")
                nc.scalar.activation(
                    sig[:], scores[:], mybir.ActivationFunctionType.Sigmoid)
                nc.sync.dma_start(t_out[r][:], sig[:])

    nc.compile()
    return nc


def _wrap_idx(idx):
    n = idx.shape[0]
    w = idx.reshape(n // 16, 16).T.astype(np.int16)
    return np.ascontiguousarray(np.tile(w, (8, 1)))


def _pack_schedule(u_local, v_idx, nblk):
    """Pack edges (sorted by u_local) into T2 chunks of 128 where chunk t may
    only use rows assigned to block blk_t = t*nblk//T2, at most 128 distinct
    rows per block.  Returns (ids, v16, edge_of_slot, vpos) or None."""
    n = u_local.shape[0]
    rows, starts, counts = np.unique(u_local, return_index=True,
                                     return_counts=True)
    nrows = rows.shape[0]
    ids = np.zeros(EL, np.uint8)
    v16 = np.zeros(EL, np.int16)         # pad slots gather row 0 (discarded)
    eos = np.full(EL, -1, np.int64)
    vpos = np.full(int(u_local[-1]) + 1, -1, np.int64)

    blk_of_t = np.array([_blk_of(t, nblk) for t in range(T2)], np.int64)
    t_first = np.searchsorted(blk_of_t, np.arange(nblk), side="left")
    t_last = np.searchsorted(blk_of_t, np.arange(nblk), side="right")
    ri = 0
    for b in range(nblk):
        cap = 128 * int(t_last[b] - t_first[b])
        slot0 = 128 * int(t_first[b])
        used = 0
        rib = 0
        while ri < nrows and rib < 128 and used + int(counts[ri]) <= cap:
            c = int(counts[ri])
            s = int(starts[ri])
            sl = slot0 + used
            ids[sl:sl + c] = rib
            v16[sl:sl + c] = v_idx[s:s + c].astype(np.int16)
            eos[sl:sl + c] = np.arange(s, s + c)
            vpos[int(rows[ri])] = 128 * b + rib
            used += c
            rib += 1
            ri += 1
    if ri != nrows:
        return None
    return ids, v16, eos, vpos


def _prepare(rels, sliced, nblk_f, nblk_r, W):
    slot_maps = [[None] * N_CORES for _ in range(6)]
    in_maps = []
    iota = np.arange(128, dtype=np.float32).reshape(128, 1)
    for c in range(N_CORES):
        m = {"iota": iota}
        for r in range(6):
            dr = int(r >= 3)
            nblk = nblk_f if dr == 0 else nblk_r
            u_local, v_idx, lo = sliced[r][c]
            packed = _pack_schedule(u_local, v_idx, nblk)
            if packed is None:
                return None, None, (r, c)
            ids, v16, eos, vpos = packed
            tab = rels[r][2]
            span = vpos.shape[0]
            urows = np.zeros((nblk * 128, D), np.float32)
            valid = vpos >= 0
            urows[vpos[valid]] = (tab[lo:lo + span][valid]
                                  * W[r][None, :]).astype(np.float32)
            m[f"u{r}"] = urows.astype(BF16)
            m[f"ids{r}"] = np.ascontiguousarray(
                np.broadcast_to(ids[None, :], (128, EL)))
            m[f"iv{r}"] = _wrap_idx(v16)
            slot_maps[r][c] = eos
        in_maps.append(m)
    return slot_maps, in_maps, None


def kernel(h_drug, h_disease, W, drug_src, dis_dst, dis_src, drug_dst):
    from concourse.bass_utils import run_bass_kernel_spmd

    h_drug = np.asarray(h_drug, dtype=np.float32)
    h_disease = np.asarray(h_disease, dtype=np.float32)
    W = np.asarray(W, dtype=np.float32)

    rels = []
    for r in range(3):
        rels.append((np.asarray(drug_src[r]), np.asarray(dis_dst[r]), h_drug))
    for r in range(3):
        rels.append((np.asarray(dis_src[r]), np.asarray(drug_dst[r]),
                     h_disease))

    perms = []
    sliced = []
    for r in range(6):
        u_idx, v_idx, _ = rels[r]
        perm = np.argsort(u_idx, kind="stable")
        perms.append(perm)
        us, vs = u_idx[perm], v_idx[perm]
        sl = []
        for c in range(N_CORES):
            ui = us[c * EPC:(c + 1) * EPC]
            vi = vs[c * EPC:(c + 1) * EPC]
            lo = int(ui[0])
            sl.append((ui - lo, vi, lo))
        sliced.append(sl)

    def span_max(dr):
        sp = 0
        for r in (range(3) if dr == 0 else range(3, 6)):
            for c in range(N_CORES):
                sp = max(sp, int(sliced[r][c][0][-1]) + 1)
        return sp

    nblk_f = max(2, -(-span_max(0) // 112))
    nblk_r = max(2, -(-span_max(1) // 112))

    slot_maps = in_maps = None
    for _attempt in range(6):
        slot_maps, in_maps, fail = _prepare(rels, sliced, nblk_f, nblk_r, W)
        if fail is None:
            break
        if fail[0] < 3:
            nblk_f += 1
        else:
            nblk_r += 1
    else:
        raise RuntimeError("could not build a feasible chunk schedule")

    hs16 = h_disease
    hd16 = h_drug
    for m in in_maps:
        m["hs"] = hs16
        m["hd"] = hd16

    cfg = (nblk_f, nblk_r, T2)
    if cfg not in _cache:
        _cache[cfg] = _build_nc(cfg)
    nc = _cache[cfg]

    res = run_bass_kernel_spmd(nc, in_maps, core_ids=list(range(N_CORES)))
    _last["exec_time_ns"] = res.exec_time_ns
    if res.instructions_and_trace is not None:
        _last["trace_path"] = res.instructions_and_trace[1]

    out = np.empty((6, E), np.float32)
    for r in range(6):
        sorted_scores = np.empty(EPC * N_CORES, np.float32)
        for c in range(N_CORES):
            s = np.asarray(res.results[c][f"scores{r}"]).astype(np.float32)
            flat = s.T.reshape(-1)                 # slot j = t*128+p
            eos = slot_maps[r][c]
            valid = eos >= 0
            sorted_scores[c * EPC + eos[valid]] = flat[valid]
        out[r, perms[r]] = sorted_scores
    return out


# revision 12
# speedup vs baseline: 1.2595x; 1.2595x over previous
"""DistMult edge scorer on 8 Trainium2 NeuronCores.

score[r, e] = sigmoid(sum_d h_u[src[r,e], d] * W[r, d] * h_v[dst[r,e], d])

Sharding: edges of each relation are sorted by source node on the host and
split into 8 contiguous slices (one per core).  All tables are bf16.

Per core, per relation, per-edge tensors live in [e(partition), d(free)]:
  - v side: per-edge rows fetched with SWDGE dma_gather (bf16 256B rows, one
    descriptor per edge, 4 big calls per relation on rotating queues; pad
    slots use index -1 so their descriptors are skipped).
  - u side: the core's rows are W-prescaled on the host and scattered into
    128-row blocks; edges are packed so chunk t of 128 edges uses rows of
    the single compile-time block blk_t = t*NB//T2.  PE expands per edge:
    ue[e,d] = matmul(lhsT=onehot[row,e], rhs=u_block[row,d]).
  - one-hots are DVE is_equal(ids, iota) in bf16 (ids host-computed u8).
  - ACT evicts the PSUM expansion to bf16; DVE scalar_tensor_tensor fuses
    the v-multiply and the d-reduction (accum_out) straight into an SBUF
    [128, T2] scoreboard; ACT applies sigmoid; host casts/unpermutes.
"""

import numpy as np
import ml_dtypes

BF16 = ml_dtypes.bfloat16

N_DRUG, N_DIS, D = 8000, 18000, 128
N_REL_DIR, E = 3, 200000
N_CORES = 8
EPC = E // N_CORES            # 25000 edges per core per relation

T2 = 224                      # chunks per (relation, core)
EL = T2 * 128                 # edge slots
NI = 1024                     # indices per gather call (8 chunks)
NCALL = EL // NI              # gather calls per relation
GRP = 8                       # chunks per group
NG = T2 // GRP                # groups per relation

_cache = {}
_last = {}


def _blk_of(t, nb):
    return t * nb // T2


def _build_nc(cfg):
    import concourse.bacc as bacc
    import concourse.mybir as mybir
    from concourse.tile import TileContext

    f32 = mybir.dt.float32
    bf16 = mybir.dt.bfloat16
    i16 = mybir.dt.int16
    u8 = mybir.dt.uint8

    nblk_f, nblk_r, _t2 = cfg
    assert _t2 == T2
    nblk = {0: nblk_f, 1: nblk_r}

    nc = bacc.Bacc("TRN2", target_bir_lowering=False, debug=False,
                   num_devices=N_CORES, num_swdge_queues=4)

    t_hd = nc.dram_tensor("hd", (N_DRUG, D), f32, kind="ExternalInput")
    t_hs = nc.dram_tensor("hs", (N_DIS, D), f32, kind="ExternalInput")
    t_u = [nc.dram_tensor(f"u{r}", (nblk[r >= 3] * 128, D), bf16,
                          kind="ExternalInput") for r in range(6)]
    t_iota = nc.dram_tensor("iota", (128, 1), f32, kind="ExternalInput")
    t_ids = [nc.dram_tensor(f"ids{r}", (128, EL), u8,
                            kind="ExternalInput") for r in range(6)]
    t_iv = [nc.dram_tensor(f"iv{r}", (128, EL // 16), i16,
                           kind="ExternalInput") for r in range(6)]
    t_out = [nc.dram_tensor(f"scores{r}", (128, T2), bf16,
                            kind="ExternalOutput") for r in range(6)]

    with TileContext(nc) as tc:
        with tc.tile_pool(name="cst", bufs=1) as cst, \
             tc.tile_pool(name="mp", bufs=2) as mp, \
             tc.tile_pool(name="gvp", bufs=7) as gvp, \
             tc.tile_pool(name="ohp", bufs=3) as ohp, \
             tc.tile_pool(name="evp", bufs=3) as evp, \
             tc.tile_pool(name="pdp", bufs=3) as pdp, \
             tc.tile_pool(name="pue", bufs=3, space="PSUM") as pue:
            iota = cst.tile([128, 1], f32)
            nc.sync.dma_start(iota[:], t_iota[:])

            swdge_calls = 0
            for r in range(6):
                dr = int(r >= 3)
                NB = nblk[dr]
                v_tab = t_hs if dr == 0 else t_hd

                u_sb = mp.tile([128, NB, D], bf16, tag=f"usb{dr}")
                nc.sync.dma_start(
                    u_sb[:], t_u[r][:].rearrange("(b p) d -> p b d", p=128))
                ids = mp.tile([128, EL], u8, tag="ids")
                nc.sync.dma_start(ids[:], t_ids[r][:])
                iv = mp.tile([128, EL // 16], i16, tag="iv")
                nc.sync.dma_start(iv[:], t_iv[r][:])

                gvs = []
                for k in range(NCALL):
                    gv = gvp.tile([128, NI // 128, D], f32, tag="gv")
                    # queue stays congruent with the scheduler's 8-wide
                    # DMASW sem round-robin (each sem is locked to a queue)
                    nc.gpsimd.dma_gather(
                        gv[:], v_tab[:],
                        iv[:, k * (NI // 16):(k + 1) * (NI // 16)],
                        NI, NI, D, elem_step=D, transpose=False,
                        single_packet=False,
                        queue_num=swdge_calls % 4)
                    swdge_calls += 1
                    gvs.append(gv)

                scores = mp.tile([128, T2], f32, tag="scores")
                ohs, ues, evs = {}, {}, {}
                for g in range(NG + 2):
                    if g < NG:
                        oh = ohp.tile([128, GRP, 128], bf16, tag="oh")
                        nc.vector.tensor_scalar(
                            oh[:].rearrange("p a b -> p (a b)"),
                            ids[:, g * 1024:(g + 1) * 1024],
                            iota[:, 0:1], None,
                            op0=mybir.AluOpType.is_equal)
                        ohs[g] = oh
                        ue = pue.tile([128, GRP, D], f32, tag="ue")
                        for c in range(GRP):
                            t = g * GRP + c
                            nc.tensor.matmul(
                                ue[:, c, :],
                                lhsT=oh[:, c, :],
                                rhs=u_sb[:, _blk_of(t, NB), :],
                                start=True, stop=True)
                        ues[g] = ue
                    if 1 <= g <= NG:
                        gp = g - 1
                        ev = evp.tile([128, GRP, D], bf16, tag="ev")
                        nc.scalar.activation(
                            ev[:].rearrange("p a b -> p (a b)"),
                            ues[gp][:].rearrange("p a b -> p (a b)"),
                            mybir.ActivationFunctionType.Copy)
                        evs[gp] = ev
                    if g >= 2:
                        gp = g - 2
                        t0 = gp * GRP
                        pd = pdp.tile([128, GRP, D], bf16, tag="pd")
                        nc.vector.tensor_tensor(
                            pd[:].rearrange("p a b -> p (a b)"),
                            evs[gp][:].rearrange("p a b -> p (a b)"),
                            gvs[gp][:, :, :].rearrange(
                                "p a b -> p (a b)"),
                            op=mybir.AluOpType.mult)
                        # tree-reduce over d: 128 -> 16 via bf16 2x adds,
                        # then one reduce_sum for the tail
                        s1 = pdp.tile([128, GRP, 64], bf16, tag="s1")
                        nc.vector.tensor_tensor(
                            s1[:], pd[:, :, :64], pd[:, :, 64:],
                            op=mybir.AluOpType.add)
                        s2 = pdp.tile([128, GRP, 32], bf16, tag="s2")
                        nc.vector.tensor_tensor(
                            s2[:], s1[:, :, :32], s1[:, :, 32:],
                            op=mybir.AluOpType.add)
                        s3 = pdp.tile([128, GRP, 16], bf16, tag="s3")
                        nc.vector.tensor_tensor(
                            s3[:], s2[:, :, :16], s2[:, :, 16:],
                            op=mybir.AluOpType.add)
                        nc.vector.reduce_sum(
                            scores[:, t0:t0 + GRP], s3[:],
                            axis=mybir.AxisListType.X)

                sig = mp.tile([128, T2], bf16, tag="sig")
                nc.scalar.activation(
                    sig[:], scores[:], mybir.ActivationFunctionType.Sigmoid)
                nc.sync.dma_start(t_out[r][:], sig[:])

    nc.compile()
    return nc


def _wrap_idx(idx):
    n = idx.shape[0]
    w = idx.reshape(n // 16, 16).T.astype(np.int16)
    return np.ascontiguousarray(np.tile(w, (8, 1)))


def _pack_schedule(u_local, v_idx, nblk):
    """Pack edges (sorted by u_local) into T2 chunks of 128 where chunk t may
    only use rows assigned to block blk_t = t*nblk//T2, at most 128 distinct
    rows per block.  Returns (ids, v16, edge_of_slot, vpos) or None."""
    n = u_local.shape[0]
    rows, starts, counts = np.unique(u_local, return_index=True,
                                     return_counts=True)
    nrows = rows.shape[0]
    ids = np.zeros(EL, np.uint8)
    v16 = np.zeros(EL, np.int16)         # pad slots gather row 0 (discarded)
    eos = np.full(EL, -1, np.int64)
    vpos = np.full(int(u_local[-1]) + 1, -1, np.int64)

    blk_of_t = np.array([_blk_of(t, nblk) for t in range(T2)], np.int64)
    t_first = np.searchsorted(blk_of_t, np.arange(nblk), side="left")
    t_last = np.searchsorted(blk_of_t, np.arange(nblk), side="right")
    ri = 0
    for b in range(nblk):
        cap = 128 * int(t_last[b] - t_first[b])
        slot0 = 128 * int(t_first[b])
        used = 0
        rib = 0
        while ri < nrows and rib < 128 and used + int(counts[ri]) <= cap:
            c = int(counts[ri])
            s = int(starts[ri])
            sl = slot0 + used
            ids[sl:sl + c] = rib
            v16[sl:sl + c] = v_idx[s:s + c].astype(np.int16)
            eos[sl:sl + c] = np.arange(s, s + c)
            vpos[int(rows[ri])] = 128 * b + rib
            used += c
            rib += 1
            ri += 1
    if ri != nrows:
        return None
    return ids, v16, eos, vpos


def _prepare(rels, sliced, nblk_f, nblk_r, W):
    slot_maps = [[None] * N_CORES for _ in range(6)]
    in_maps = []
    iota = np.arange(128, dtype=np.float32).reshape(128, 1)
    for c in range(N_CORES):
        m = {"iota": iota}
        for r in range(6):
            dr = int(r >= 3)
            nblk = nblk_f if dr == 0 else nblk_r
            u_local, v_idx, lo = sliced[r][c]
            packed = _pack_schedule(u_local, v_idx, nblk)
            if packed is None:
                return None, None, (r, c)
            ids, v16, eos, vpos = packed
            tab = rels[r][2]
            span = vpos.shape[0]
            urows = np.zeros((nblk * 128, D), np.float32)
            valid = vpos >= 0
            urows[vpos[valid]] = (tab[lo:lo + span][valid]
                                  * W[r][None, :]).astype(np.float32)
            m[f"u{r}"] = urows.astype(BF16)
            m[f"ids{r}"] = np.ascontiguousarray(
                np.broadcast_to(ids[None, :], (128, EL)))
            m[f"iv{r}"] = _wrap_idx(v16)
            slot_maps[r][c] = eos
        in_maps.append(m)
    return slot_maps, in_maps, None


def kernel(h_drug, h_disease, W, drug_src, dis_dst, dis_src, drug_dst):
    from concourse.bass_utils import run_bass_kernel_spmd

    h_drug = np.asarray(h_drug, dtype=np.float32)
    h_disease = np.asarray(h_disease, dtype=np.float32)
    W = np.asarray(W, dtype=np.float32)

    rels = []
    for r in range(3):
        rels.append((np.asarray(drug_src[r]), np.asarray(dis_dst[r]), h_drug))
    for r in range(3):
        rels.append((np.asarray(dis_src[r]), np.asarray(drug_dst[r]),
                     h_disease))

    perms = []
    sliced = []
    for r in range(6):
        u_idx, v_idx, _ = rels[r]
        perm = np.argsort(u_idx, kind="stable")
        perms.append(perm)
        us, vs = u_idx[perm], v_idx[perm]
        sl = []
        for c in range(N_CORES):
            ui = us[c * EPC:(c + 1) * EPC]
            vi = vs[c * EPC:(c + 1) * EPC]
            lo = int(ui[0])
            sl.append((ui - lo, vi, lo))
        sliced.append(sl)

    def span_max(dr):
        sp = 0
        for r in (range(3) if dr == 0 else range(3, 6)):
            for c in range(N_CORES):
                sp = max(sp, int(sliced[r][c][0][-1]) + 1)
        return sp

    nblk_f = max(2, -(-span_max(0) // 112))
    nblk_r = max(2, -(-span_max(1) // 112))

    slot_maps = in_maps = None
    for _attempt in range(6):
        slot_maps, in_maps, fail = _prepare(rels, sliced, nblk_f, nblk_r, W)
        if fail is None:
            break
        if fail[0] < 3:
            nblk_f += 1
        else:
            nblk_r += 1
    else:
        raise RuntimeError("could not build a feasible chunk schedule")

    hs16 = h_disease
    hd16 = h_drug
    for m in in_maps:
        m["hs"] = hs16
        m["hd"] = hd16

    cfg = (nblk_f, nblk_r, T2)
    if cfg not in _cache:
        _cache[cfg] = _build_nc(cfg)
    nc = _cache[cfg]

    res = run_bass_kernel_spmd(nc, in_maps, core_ids=list(range(N_CORES)))
    _last["exec_time_ns"] = res.exec_time_ns
    if res.instructions_and_trace is not None:
        _last["trace_path"] = res.instructions_and_trace[1]

    out = np.empty((6, E), np.float32)
    for r in range(6):
        sorted_scores = np.empty(EPC * N_CORES, np.float32)
        for c in range(N_CORES):
            s = np.asarray(res.results[c][f"scores{r}"]).astype(np.float32)
            flat = s.T.reshape(-1)                 # slot j = t*128+p
            eos = slot_maps[r][c]
            valid = eos >= 0
            sorted_scores[c * EPC + eos[valid]] = flat[valid]
        out[r, perms[r]] = sorted_scores
    return out


# revision 13
# speedup vs baseline: 2.0954x; 1.6637x over previous
"""DistMult edge scorer on 8 Trainium2 NeuronCores.

score[r, e] = sigmoid(sum_d h_u[src[r,e], d] * W[r, d] * h_v[dst[r,e], d])

Sharding: edges of each relation are sorted by source node on the host and
split into 8 contiguous slices (one per core).

Per core, per relation:
  - u side: the core's contiguous source-row range is DMA'd into SBUF once,
    prescaled by W[r] (DVE), and expanded per edge by PE one-hot selection
    matmuls.  Chunk t of 128 edges may only use source rows inside a
    two-block window [128*B_t, 128*(B_t+2)) where B_t = floor(t*NBLK/T2) is
    compile-time; the host packs edges greedily into chunks under that
    constraint (uniform data tracks the linear schedule closely).
  - v side: per-edge rows fetched with SWDGE dma_gather (512B rows,
    edges-on-partitions).  This is the bottleneck: the gather ucode costs
    ~8 ns per index on the Pool engine regardless of elem size.
  - DVE builds the one-hot masks (iota==ids) and does multiply+reduce;
    ACT applies sigmoid; scores are DMA'd out contiguously and unpermuted
    on the host.
"""

import numpy as np

N_DRUG, N_DIS, D = 8000, 18000, 128
N_REL_DIR, E = 3, 200000
N_CORES = 8
EPC = E // N_CORES          # 25000 edges per core per relation
T2 = 200                    # chunks per (relation, core); 25600 edge slots
EL = T2 * 128

_cache = {}
_last = {}


def _blk_of(t, nb):
    return min(t * (nb - 1) // T2, nb - 2)


def _build_nc(cfg):
    import concourse.bacc as bacc
    import concourse.mybir as mybir
    from concourse.tile import TileContext

    f32 = mybir.dt.float32
    i16 = mybir.dt.int16
    u8 = mybir.dt.uint8

    nblk_f, nblk_r, _t2 = cfg
    assert _t2 == T2
    nblk = {0: nblk_f, 1: nblk_r}

    nc = bacc.Bacc("TRN2", target_bir_lowering=False, debug=False,
                   num_devices=N_CORES, num_swdge_queues=4)

    t_hd = nc.dram_tensor("hd", (N_DRUG, D), f32, kind="ExternalInput")
    t_hs = nc.dram_tensor("hs", (N_DIS, D), f32, kind="ExternalInput")
    t_u = [nc.dram_tensor(f"u{r}", (nblk[r >= 3] * 128, D), f32,
                          kind="ExternalInput") for r in range(6)]
    t_wb = nc.dram_tensor("wb", (128, 6, D), f32, kind="ExternalInput")
    t_iota = nc.dram_tensor("iota", (128, 2), u8, kind="ExternalInput")
    t_ids = [nc.dram_tensor(f"ids{r}", (128, EL), u8,
                            kind="ExternalInput") for r in range(6)]
    t_iv = [nc.dram_tensor(f"iv{r}", (128, EL // 16), i16,
                           kind="ExternalInput") for r in range(6)]
    t_out = [nc.dram_tensor(f"scores{r}", (128, T2), f32,
                            kind="ExternalOutput") for r in range(6)]
    t_iu = [nc.dram_tensor(f"iu{r}", (128, EL // 16), i16,
                           kind="ExternalInput") for r in range(6)]
    t_us = [nc.dram_tensor(f"us{r}", (nblk[r >= 3] * 128, D), f32,
                           kind="Internal") for r in range(6)]

    with TileContext(nc) as tc:
        with tc.tile_pool(name="cst", bufs=1) as cst, \
             tc.tile_pool(name="mp", bufs=2) as mp, \
             tc.tile_pool(name="gp", bufs=2) as gp, \
             tc.tile_pool(name="gvp", bufs=3) as gvp, \
             tc.tile_pool(name="pp", bufs=4, space="PSUM") as pp:
            wb = cst.tile([128, 6, D], f32)
            iota = cst.tile([128, 2], u8)
            nc.sync.dma_start(wb[:], t_wb[:])
            nc.sync.dma_start(iota[:], t_iota[:])
            for r in range(6):
                dr = int(r >= 3)
                NB = nblk[dr]
                v_tab = t_hs if dr == 0 else t_hd

                # u range -> SBUF (row 128b+p at [p, b, :]), prescale by W[r]
                u_sb = mp.tile([128, NB, D], f32, tag=f"usb{dr}")
                nc.sync.dma_start(
                    u_sb[:], t_u[r][:].rearrange("(b p) d -> p b d", p=128))
                for b in range(NB):
                    nc.vector.tensor_tensor(
                        u_sb[:, b, :], u_sb[:, b, :], wb[:, r, :],
                        op=mybir.AluOpType.mult)
                # scaled copy to DRAM scratch for the gathered-u chunks
                nc.sync.dma_start(
                    t_us[r][:].rearrange("(b p) d -> p b d", p=128), u_sb[:])

                iv = mp.tile([128, EL // 16], i16, tag="iv")
                nc.sync.dma_start(iv[:], t_iv[r][:])
                iu = mp.tile([128, EL // 16], i16, tag="iu")
                nc.sync.dma_start(iu[:], t_iu[r][:])
                scores = mp.tile([128, T2], f32, tag="scores")

                batches = [40] * (T2 // 40) + ([T2 % 40] if T2 % 40 else [])
                c0 = 0
                for b, kbn in enumerate(batches):
                    nb_i = kbn * 128
                    gv = gvp.tile([128, 40, D], f32, tag="gv")
                    # split each batch across the 4 SWDGE queues: desc-gen for
                    # queue q runs on Q7 core pair q, so the four quarters
                    # generate concurrently
                    # the queue that also carries this batch's u-gather gets
                    # a smaller v share so per-pair desc-gen is balanced
                    # fine-grained, pair-balanced issue: pairs 0/1 take
                    # 2x7 v-chunks, pairs 2/3 take 6 v-chunks (they also
                    # carry the 8-chunk u-gathers) -> 14 chunks per pair
                    gx = min(16, ((2 * kbn) // 5) & ~3)
                    gu = gp.tile([128, 16, D], f32, tag="gu")
                    gh = gx // 2
                    if kbn == 40:
                        segs = [(0, 7), (1, 7), (2, 6), (3, 6),
                                ("u", 2), ("u2", 3), (0, 7), (1, 7)]
                    else:
                        qn = max(1, kbn // 4)
                        segs = []
                        left, q = kbn, 0
                        while left > 0:
                            take = min(qn, left)
                            segs.append((q % 4, take))
                            left -= take
                            q += 1
                        if gx > 0:
                            segs += [("u", 2), ("u2", 3)]
                    k0 = 0
                    for q, sz in segs:
                        if q == "u":
                            # u-gathers mid-wave so pairs 2/3 keep working
                            # while the engine stalls on pairs 0/1
                            nc.gpsimd.dma_gather(
                                gu[:, :gh, :], t_us[r][:],
                                iu[:, c0 * 8:(c0 + gh) * 8],
                                gh * 128, gh * 128, D, elem_step=D,
                                single_packet=False, queue_num=sz)
                            continue
                        if q == "u2":
                            nc.gpsimd.dma_gather(
                                gu[:, gh:gx, :], t_us[r][:],
                                iu[:, (c0 + gh) * 8:(c0 + gx) * 8],
                                (gx - gh) * 128, (gx - gh) * 128, D,
                                elem_step=D, single_packet=False,
                                queue_num=sz)
                            continue
                        k1 = k0 + sz
                        nc.gpsimd.dma_gather(
                            gv[:, k0:k1, :], v_tab[:],
                            iv[:, (c0 + k0) * 8:(c0 + k1) * 8],
                            sz * 128, sz * 128, D,
                            elem_step=D, single_packet=False, queue_num=q)
                        k0 = k1
                    noh = kbn - gx
                    ids = gp.tile([128, 24 * 128], u8, tag="ids")
                    nc.sync.dma_start(
                        ids[:, :noh * 128],
                        t_ids[r][:, (c0 + gx) * 128:(c0 + kbn) * 128])
                    oh_lo = gp.tile([128, 24 * 128], f32, tag="ohlo")
                    oh_hi = gp.tile([128, 24 * 128], f32, tag="ohhi")
                    nc.vector.tensor_tensor(
                        oh_lo[:, :noh * 128], ids[:, :noh * 128],
                        iota[:, 0:1].to_broadcast([128, noh * 128]),
                        op=mybir.AluOpType.is_equal)
                    nc.vector.tensor_tensor(
                        oh_hi[:, :noh * 128], ids[:, :noh * 128],
                        iota[:, 1:2].to_broadcast([128, noh * 128]),
                        op=mybir.AluOpType.is_equal)
                    for g0 in range(0, kbn, 4):
                        gn = min(4, kbn - g0)
                        if g0 + gn <= gx:
                            usrc = gu[:, g0:g0 + gn, :]
                        elif g0 >= gx:
                            ps = pp.tile([128, 4, D], f32, tag="ps")
                            for i in range(g0, g0 + gn):
                                t = c0 + i
                                blk = _blk_of(t, NB)
                                j = i - gx
                                nc.tensor.matmul(
                                    ps[:, i - g0, :],
                                    lhsT=oh_lo[:, j * 128:(j + 1) * 128],
                                    rhs=u_sb[:, blk, :],
                                    start=True, stop=False)
                                nc.tensor.matmul(
                                    ps[:, i - g0, :],
                                    lhsT=oh_hi[:, j * 128:(j + 1) * 128],
                                    rhs=u_sb[:, blk + 1, :],
                                    start=False, stop=True)
                            usrc = ps[:, :gn, :]
                        else:
                            raise AssertionError("gx must be multiple of 4")
                        prod = gp.tile([128, 4, D], f32, tag="prod")
                        nc.vector.tensor_tensor(
                            prod[:, :gn, :].rearrange("p a b -> p (a b)"),
                            usrc.rearrange("p a b -> p (a b)"),
                            gv[:, g0:g0 + gn, :].rearrange("p a b -> p (a b)"),
                            op=mybir.AluOpType.mult)
                        # reduction split between scalar engine (4x slower
                        # per chunk but otherwise idle) and DVE
                        if (g0 // 4) % 3 == 0:
                            nc.vector.reduce_sum(
                                scores[:, c0 + g0:c0 + g0 + gn],
                                prod[:, :gn, :], axis=mybir.AxisListType.X)
                        else:
                            acts = cst.tile([128, D], f32, tag="actout")
                            for i in range(gn):
                                nc.scalar.activation(
                                    acts[:], prod[:, i, :],
                                    mybir.ActivationFunctionType.Copy,
                                    accum_out=scores[:, c0 + g0 + i:c0 + g0 + i + 1])
                    c0 += kbn

                sig = mp.tile([128, T2], f32, tag="sig")
                nc.scalar.activation(
                    sig[:], scores[:], mybir.ActivationFunctionType.Sigmoid)
                nc.sync.dma_start(t_out[r][:], sig[:])

    nc.compile()
    return nc


def _wrap_idx(idx):
    n = idx.shape[0]
    w = idx.reshape(n // 16, 16).T.astype(np.int16)
    return np.ascontiguousarray(np.tile(w, (8, 1)))


def _pack_schedule(u_local, v_idx, nblk):
    """Greedy pack sorted edges into T2 chunks of 128 under the two-block
    window [128*B_t, 128*(B_t+2)).  Returns (ids_u8, v16, slot_of_edge)."""
    n = u_local.shape[0]
    ids = np.zeros(EL, np.uint8)
    v16 = np.zeros(EL, np.int16)
    edge_of_slot = np.full(EL, -1, np.int64)
    ptr = 0
    for t in range(T2):
        bt = min(t * (nblk - 1) // T2, nblk - 2)
        lo_row, hi_row = 128 * bt, 128 * (bt + 2)
        if ptr < n and u_local[ptr] < lo_row:
            raise RuntimeError("schedule fell behind data")
        # edges are sorted; find how many fit this window
        hi = np.searchsorted(u_local, hi_row, side="left")
        take = min(128, hi - ptr)
        if take > 0:
            s0 = t * 128
            ids[s0:s0 + take] = (u_local[ptr:ptr + take] - lo_row).astype(np.uint8)
            v16[s0:s0 + take] = v_idx[ptr:ptr + take].astype(np.int16)
            edge_of_slot[s0:s0 + take] = np.arange(ptr, ptr + take)
            # dummy slots replicate window base row with v=0 (harmless)
            ptr += take
    if ptr != n:
        raise RuntimeError(f"schedule failed to place all edges ({ptr}/{n})")
    return ids, v16, edge_of_slot


def _prepare(rels, sliced, nblk_f, nblk_r, wb, iota, h_drug, h_disease):
    slot_maps = [[None] * N_CORES for _ in range(6)]
    in_maps = []
    for c in range(N_CORES):
        m = {"hd": h_drug, "hs": h_disease, "wb": wb, "iota": iota}
        for r in range(6):
            dr = int(r >= 3)
            nblk = nblk_f if dr == 0 else nblk_r
            u_local, v_idx, lo = sliced[r][c]
            # Remap this core's rows to virtual rows spread by edge-count CDF
            # over [0, 128*(nblk-1)), so the data tracks the shared linear
            # chunk->block schedule exactly on every core.
            span = int(u_local[-1]) + 1
            V = 128 * (nblk - 1)
            counts = np.bincount(u_local, minlength=span).astype(np.int64)
            cum = np.concatenate([[0], np.cumsum(counts)[:-1]])
            target = (cum * V) // max(int(counts.sum()), 1)
            # strictly increasing: vpos[j] = max(target[j], vpos[j-1]+1)
            vpos = np.maximum.accumulate(target - np.arange(span)) + np.arange(span)
            if not vpos[-1] < nblk * 128:
                raise RuntimeError("virtual row remap overflow")
            u_virt = vpos[u_local]
            ids, v16, edge_of_slot = _pack_schedule(u_virt, v_idx, nblk)
            nrows = nblk * 128
            tab = rels[r][2]
            urows = np.zeros((nrows, D), np.float32)
            nn = min(span, tab.shape[0] - lo)
            urows[vpos[:nn]] = tab[lo:lo + nn]
            m[f"u{r}"] = urows
            m[f"ids{r}"] = np.ascontiguousarray(
                np.broadcast_to(ids[None, :], (128, EL)))
            m[f"iv{r}"] = _wrap_idx(v16)
            blk_arr = np.array([_blk_of(t, nblk) for t in range(T2)], np.int64)
            iu16 = (np.repeat(blk_arr, 128) * 128
                    + ids.astype(np.int64)).astype(np.int16)
            m[f"iu{r}"] = _wrap_idx(iu16)
            slot_maps[r][c] = edge_of_slot
        in_maps.append(m)
    return slot_maps, in_maps


def kernel(h_drug, h_disease, W, drug_src, dis_dst, dis_src, drug_dst):
    from concourse.bass_utils import run_bass_kernel_spmd

    h_drug = np.asarray(h_drug, dtype=np.float32)
    h_disease = np.asarray(h_disease, dtype=np.float32)
    W = np.asarray(W, dtype=np.float32)

    rels = []
    for r in range(3):
        rels.append((np.asarray(drug_src[r]), np.asarray(dis_dst[r]), h_drug))
    for r in range(3):
        rels.append((np.asarray(dis_src[r]), np.asarray(drug_dst[r]), h_disease))

    perms = []
    sliced = []
    for r in range(6):
        u_idx, v_idx, _ = rels[r]
        perm = np.argsort(u_idx, kind="stable")
        perms.append(perm)
        us, vs = u_idx[perm], v_idx[perm]
        sl = []
        for c in range(N_CORES):
            ui = us[c * EPC:(c + 1) * EPC]
            vi = vs[c * EPC:(c + 1) * EPC]
            lo = int(ui[0])
            sl.append((ui - lo, vi, lo))
        sliced.append(sl)

    def max_blocks(dr):
        nb = 2
        for r in (range(3) if dr == 0 else range(3, 6)):
            for c in range(N_CORES):
                u_local = sliced[r][c][0]
                nb = max(nb, int(u_local[-1]) // 128 + 2)
        return nb

    nblk_f, nblk_r = max_blocks(0), max_blocks(1)

    wb = np.ascontiguousarray(np.broadcast_to(W[None, :, :], (128, 6, D)),
                              dtype=np.float32)
    iota = np.empty((128, 2), np.uint8)
    iota[:, 0] = np.arange(128)
    iota[:, 1] = np.arange(128, 256)

    global T2, EL
    for _attempt in range(4):
        try:
            slot_maps, in_maps = _prepare(rels, sliced, nblk_f, nblk_r,
                                          wb, iota, h_drug, h_disease)
            break
        except RuntimeError:
            # pathological row distribution: give the schedule more slack
            T2 += 8
            EL = T2 * 128
    else:
        raise RuntimeError("could not build a feasible chunk schedule")

    cfg = (nblk_f, nblk_r, T2)
    if cfg not in _cache:
        _cache[cfg] = _build_nc(cfg)
    nc = _cache[cfg]

    res = run_bass_kernel_spmd(nc, in_maps, core_ids=list(range(N_CORES)))
    _last["exec_time_ns"] = res.exec_time_ns
    if res.instructions_and_trace is not None:
        _last["trace_path"] = res.instructions_and_trace[1]

    out = np.empty((6, E), np.float32)
    for r in range(6):
        sorted_scores = np.empty(EPC * N_CORES, np.float32)
        for c in range(N_CORES):
            s = res.results[c][f"scores{r}"]       # [128, T2]
            flat = s.T.reshape(-1)                 # slot j = t*128+p
            eos = slot_maps[r][c]
            valid = eos >= 0
            sorted_scores[c * EPC + eos[valid]] = flat[valid]
        out[r, perms[r]] = sorted_scores
    return out



# revision 14
# speedup vs baseline: 2.2913x; 1.0935x over previous
"""DistMult edge scorer on 8 Trainium2 NeuronCores.

score[r, e] = sigmoid(sum_d h_u[src[r,e], d] * W[r, d] * h_v[dst[r,e], d])

Sharding: edges of each relation are sorted by source node on the host and
split into 8 contiguous slices (one per core).

Per core, per relation:
  - u side: the core's contiguous source-row range is DMA'd into SBUF once,
    prescaled by W[r] (DVE), and expanded per edge by PE one-hot selection
    matmuls.  Chunk t of 128 edges may only use source rows inside a
    two-block window [128*B_t, 128*(B_t+2)) where B_t = floor(t*NBLK/T2) is
    compile-time; the host packs edges greedily into chunks under that
    constraint (uniform data tracks the linear schedule closely).
  - v side: per-edge rows fetched with SWDGE dma_gather (512B rows,
    edges-on-partitions).  This is the bottleneck: the gather ucode costs
    ~8 ns per index on the Pool engine regardless of elem size.
  - DVE builds the one-hot masks (iota==ids) and does multiply+reduce;
    ACT applies sigmoid; scores are DMA'd out contiguously and unpermuted
    on the host.
"""

import numpy as np
import ml_dtypes

BF16 = ml_dtypes.bfloat16

N_DRUG, N_DIS, D = 8000, 18000, 128
N_REL_DIR, E = 3, 200000
N_CORES = 8
EPC = E // N_CORES          # 25000 edges per core per relation
T2 = 200                    # chunks per (relation, core); 25600 edge slots
EL = T2 * 128

_cache = {}
_last = {}


def _blk_of(t, nb):
    return min(t * (nb - 1) // T2, nb - 2)


def _build_nc(cfg):
    import concourse.bacc as bacc
    import concourse.mybir as mybir
    from concourse.tile import TileContext

    f32 = mybir.dt.float32
    bf16 = mybir.dt.bfloat16
    i16 = mybir.dt.int16
    u8 = mybir.dt.uint8

    nblk_f, nblk_r, _t2 = cfg
    assert _t2 == T2
    nblk = {0: nblk_f, 1: nblk_r}

    nc = bacc.Bacc("TRN2", target_bir_lowering=False, debug=False,
                   num_devices=N_CORES, num_swdge_queues=4)

    t_hd = nc.dram_tensor("hd", (N_DRUG, D), f32, kind="ExternalInput")
    t_hs = nc.dram_tensor("hs", (N_DIS, D), f32, kind="ExternalInput")
    t_u = [nc.dram_tensor(f"u{r}", (nblk[r >= 3] * 128, D), bf16,
                          kind="ExternalInput") for r in range(6)]
    t_iota = nc.dram_tensor("iota", (128, 2), f32, kind="ExternalInput")
    t_ids = [nc.dram_tensor(f"ids{r}", (128, EL), u8,
                            kind="ExternalInput") for r in range(6)]
    t_iv = [nc.dram_tensor(f"iv{r}", (128, EL // 16), i16,
                           kind="ExternalInput") for r in range(6)]
    t_out = [nc.dram_tensor(f"scores{r}", (128, T2), f32,
                            kind="ExternalOutput") for r in range(6)]

    with TileContext(nc) as tc:
        with tc.tile_pool(name="cst", bufs=1) as cst, \
             tc.tile_pool(name="mp", bufs=2) as mp, \
             tc.tile_pool(name="gp", bufs=2) as gp, \
             tc.tile_pool(name="gvp", bufs=3) as gvp, \
             tc.tile_pool(name="pp", bufs=4, space="PSUM") as pp:
            iota = cst.tile([128, 2], f32)
            nc.sync.dma_start(iota[:], t_iota[:])
            for r in range(6):
                dr = int(r >= 3)
                NB = nblk[dr]
                v_tab = t_hs if dr == 0 else t_hd

                # u range -> SBUF (row 128b+p at [p, b, :]); W-prescaled
                # bf16 on the host
                u_sb = mp.tile([128, NB, D], bf16, tag=f"usb{dr}")
                nc.sync.dma_start(
                    u_sb[:], t_u[r][:].rearrange("(b p) d -> p b d", p=128))

                iv = mp.tile([128, EL // 16], i16, tag="iv")
                nc.sync.dma_start(iv[:], t_iv[r][:])
                scores = mp.tile([128, T2], f32, tag="scores")

                batches = [40] * (T2 // 40) + ([T2 % 40] if T2 % 40 else [])
                c0 = 0
                for b, kbn in enumerate(batches):
                    nb_i = kbn * 128
                    gv = gvp.tile([128, 40, D], f32, tag="gv")
                    # split each batch across the 4 SWDGE queues: desc-gen for
                    # queue q runs on Q7 core pair q, so the four quarters
                    # generate concurrently
                    # the queue that also carries this batch's u-gather gets
                    # a smaller v share so per-pair desc-gen is balanced
                    # fine-grained, pair-balanced issue: pairs 0/1 take
                    # 2x7 v-chunks, pairs 2/3 take 6 v-chunks (they also
                    # carry the 8-chunk u-gathers) -> 14 chunks per pair
                    gx = 0
                    qn = max(1, -(-kbn // 4))
                    segs = []
                    left, q = kbn, 0
                    while left > 0:
                        take = min(qn, left)
                        segs.append((q % 4, take))
                        left -= take
                        q += 1
                    k0 = 0
                    for q, sz in segs:
                        k1 = k0 + sz
                        nc.gpsimd.dma_gather(
                            gv[:, k0:k1, :], v_tab[:],
                            iv[:, (c0 + k0) * 8:(c0 + k1) * 8],
                            sz * 128, sz * 128, D,
                            elem_step=D, single_packet=False, queue_num=q)
                        k0 = k1
                    noh = kbn
                    ids = gp.tile([128, 40 * 128], u8, tag="ids")
                    nc.sync.dma_start(
                        ids[:, :noh * 128],
                        t_ids[r][:, c0 * 128:(c0 + kbn) * 128])
                    oh_lo = gp.tile([128, 40 * 128], bf16, tag="ohlo")
                    oh_hi = gp.tile([128, 40 * 128], bf16, tag="ohhi")
                    nc.vector.tensor_scalar(
                        oh_lo[:, :noh * 128], ids[:, :noh * 128],
                        iota[:, 0:1], None, op0=mybir.AluOpType.is_equal)
                    nc.vector.tensor_scalar(
                        oh_hi[:, :noh * 128], ids[:, :noh * 128],
                        iota[:, 1:2], None, op0=mybir.AluOpType.is_equal)
                    for g0 in range(0, kbn, 4):
                        gn = min(4, kbn - g0)
                        if g0 + gn <= gx:
                            usrc = gu[:, g0:g0 + gn, :]
                        elif g0 >= gx:
                            ps = pp.tile([128, 4, D], f32, tag="ps")
                            for i in range(g0, g0 + gn):
                                t = c0 + i
                                blk = _blk_of(t, NB)
                                j = i - gx
                                nc.tensor.matmul(
                                    ps[:, i - g0, :],
                                    lhsT=oh_lo[:, j * 128:(j + 1) * 128],
                                    rhs=u_sb[:, blk, :],
                                    start=True, stop=False)
                                nc.tensor.matmul(
                                    ps[:, i - g0, :],
                                    lhsT=oh_hi[:, j * 128:(j + 1) * 128],
                                    rhs=u_sb[:, blk + 1, :],
                                    start=False, stop=True)
                            usrc = ps[:, :gn, :]
                        else:
                            raise AssertionError("gx must be multiple of 4")
                        prod = gp.tile([128, 4, D], f32, tag="prod")
                        nc.vector.tensor_tensor(
                            prod[:, :gn, :].rearrange("p a b -> p (a b)"),
                            usrc.rearrange("p a b -> p (a b)"),
                            gv[:, g0:g0 + gn, :].rearrange("p a b -> p (a b)"),
                            op=mybir.AluOpType.mult)
                        # reduction split between scalar engine (4x slower
                        # per chunk but otherwise idle) and DVE
                        if (g0 // 4) % 3 != 0:
                            nc.vector.reduce_sum(
                                scores[:, c0 + g0:c0 + g0 + gn],
                                prod[:, :gn, :], axis=mybir.AxisListType.X)
                        else:
                            acts = cst.tile([128, D], f32, tag="actout")
                            for i in range(gn):
                                nc.scalar.activation(
                                    acts[:], prod[:, i, :],
                                    mybir.ActivationFunctionType.Copy,
                                    accum_out=scores[:, c0 + g0 + i:c0 + g0 + i + 1])
                    c0 += kbn

                sig = mp.tile([128, T2], f32, tag="sig")
                nc.scalar.activation(
                    sig[:], scores[:], mybir.ActivationFunctionType.Sigmoid)
                nc.sync.dma_start(t_out[r][:], sig[:])

    nc.compile()
    return nc


def _wrap_idx(idx):
    n = idx.shape[0]
    w = idx.reshape(n // 16, 16).T.astype(np.int16)
    return np.ascontiguousarray(np.tile(w, (8, 1)))


def _pack_schedule(u_local, v_idx, nblk):
    """Greedy pack sorted edges into T2 chunks of 128 under the two-block
    window [128*B_t, 128*(B_t+2)).  Returns (ids_u8, v16, slot_of_edge)."""
    n = u_local.shape[0]
    ids = np.zeros(EL, np.uint8)
    v16 = np.zeros(EL, np.int16)
    edge_of_slot = np.full(EL, -1, np.int64)
    ptr = 0
    for t in range(T2):
        bt = min(t * (nblk - 1) // T2, nblk - 2)
        lo_row, hi_row = 128 * bt, 128 * (bt + 2)
        if ptr < n and u_local[ptr] < lo_row:
            raise RuntimeError("schedule fell behind data")
        # edges are sorted; find how many fit this window
        hi = np.searchsorted(u_local, hi_row, side="left")
        take = min(128, hi - ptr)
        if take > 0:
            s0 = t * 128
            ids[s0:s0 + take] = (u_local[ptr:ptr + take] - lo_row).astype(np.uint8)
            v16[s0:s0 + take] = v_idx[ptr:ptr + take].astype(np.int16)
            edge_of_slot[s0:s0 + take] = np.arange(ptr, ptr + take)
            # dummy slots replicate window base row with v=0 (harmless)
            ptr += take
    if ptr != n:
        raise RuntimeError(f"schedule failed to place all edges ({ptr}/{n})")
    return ids, v16, edge_of_slot


def _prepare(rels, sliced, nblk_f, nblk_r, W, iota, h_drug, h_disease):
    slot_maps = [[None] * N_CORES for _ in range(6)]
    in_maps = []
    for c in range(N_CORES):
        m = {"hd": h_drug, "hs": h_disease, "iota": iota}
        for r in range(6):
            dr = int(r >= 3)
            nblk = nblk_f if dr == 0 else nblk_r
            u_local, v_idx, lo = sliced[r][c]
            # Remap this core's rows to virtual rows spread by edge-count CDF
            # over [0, 128*(nblk-1)), so the data tracks the shared linear
            # chunk->block schedule exactly on every core.
            span = int(u_local[-1]) + 1
            V = 128 * (nblk - 1)
            counts = np.bincount(u_local, minlength=span).astype(np.int64)
            cum = np.concatenate([[0], np.cumsum(counts)[:-1]])
            target = (cum * V) // max(int(counts.sum()), 1)
            # strictly increasing: vpos[j] = max(target[j], vpos[j-1]+1)
            vpos = np.maximum.accumulate(target - np.arange(span)) + np.arange(span)
            if not vpos[-1] < nblk * 128:
                raise RuntimeError("virtual row remap overflow")
            u_virt = vpos[u_local]
            ids, v16, edge_of_slot = _pack_schedule(u_virt, v_idx, nblk)
            nrows = nblk * 128
            tab = rels[r][2]
            urows = np.zeros((nrows, D), np.float32)
            nn = min(span, tab.shape[0] - lo)
            urows[vpos[:nn]] = tab[lo:lo + nn] * W[r][None, :]
            m[f"u{r}"] = urows.astype(BF16)
            m[f"ids{r}"] = np.ascontiguousarray(
                np.broadcast_to(ids[None, :], (128, EL)))
            m[f"iv{r}"] = _wrap_idx(v16)
            slot_maps[r][c] = edge_of_slot
        in_maps.append(m)
    return slot_maps, in_maps


def kernel(h_drug, h_disease, W, drug_src, dis_dst, dis_src, drug_dst):
    from concourse.bass_utils import run_bass_kernel_spmd

    h_drug = np.asarray(h_drug, dtype=np.float32)
    h_disease = np.asarray(h_disease, dtype=np.float32)
    W = np.asarray(W, dtype=np.float32)

    rels = []
    for r in range(3):
        rels.append((np.asarray(drug_src[r]), np.asarray(dis_dst[r]), h_drug))
    for r in range(3):
        rels.append((np.asarray(dis_src[r]), np.asarray(drug_dst[r]), h_disease))

    perms = []
    sliced = []
    for r in range(6):
        u_idx, v_idx, _ = rels[r]
        perm = np.argsort(u_idx, kind="stable")
        perms.append(perm)
        us, vs = u_idx[perm], v_idx[perm]
        sl = []
        for c in range(N_CORES):
            ui = us[c * EPC:(c + 1) * EPC]
            vi = vs[c * EPC:(c + 1) * EPC]
            lo = int(ui[0])
            sl.append((ui - lo, vi, lo))
        sliced.append(sl)

    def max_blocks(dr):
        nb = 2
        for r in (range(3) if dr == 0 else range(3, 6)):
            for c in range(N_CORES):
                u_local = sliced[r][c][0]
                nb = max(nb, int(u_local[-1]) // 128 + 2)
        return nb

    nblk_f, nblk_r = max_blocks(0), max_blocks(1)

    iota = np.empty((128, 2), np.float32)
    iota[:, 0] = np.arange(128)
    iota[:, 1] = np.arange(128, 256)

    global T2, EL
    for _attempt in range(4):
        try:
            slot_maps, in_maps = _prepare(rels, sliced, nblk_f, nblk_r,
                                          W, iota, h_drug, h_disease)
            break
        except RuntimeError:
            # pathological row distribution: give the schedule more slack
            T2 += 8
            EL = T2 * 128
    else:
        raise RuntimeError("could not build a feasible chunk schedule")

    cfg = (nblk_f, nblk_r, T2)
    if cfg not in _cache:
        _cache[cfg] = _build_nc(cfg)
    nc = _cache[cfg]

    res = run_bass_kernel_spmd(nc, in_maps, core_ids=list(range(N_CORES)))
    _last["exec_time_ns"] = res.exec_time_ns
    if res.instructions_and_trace is not None:
        _last["trace_path"] = res.instructions_and_trace[1]

    out = np.empty((6, E), np.float32)
    for r in range(6):
        sorted_scores = np.empty(EPC * N_CORES, np.float32)
        for c in range(N_CORES):
            s = res.results[c][f"scores{r}"]       # [128, T2]
            flat = s.T.reshape(-1)                 # slot j = t*128+p
            eos = slot_maps[r][c]
            valid = eos >= 0
            sorted_scores[c * EPC + eos[valid]] = flat[valid]
        out[r, perms[r]] = sorted_scores
    return out



# revision 15
# speedup vs baseline: 2.2927x; 1.0006x over previous
"""DistMult edge scorer on 8 Trainium2 NeuronCores.

score[r, e] = sigmoid(sum_d h_u[src[r,e], d] * W[r, d] * h_v[dst[r,e], d])

Sharding: edges of each relation are sorted by source node on the host and
split into 8 contiguous slices (one per core).

Per core, per relation:
  - u side: the core's contiguous source-row range is W-prescaled on the
    host, cast to bf16, DMA'd into SBUF, and expanded per edge by bf16 PE
    one-hot matmuls (no u gathers).  Chunk t of 128 edges may only use
    source rows inside a two-block window [128*B_t, 128*(B_t+2)) with
    B_t = floor(t*NBLK/T2) fixed at compile time; the host packs edges
    greedily into chunks under that constraint.
  - v side: per-edge f32 rows fetched with SWDGE dma_gather (512B rows,
    edges-on-partitions), 4 sem-congruent calls per 40-chunk batch.  The
    gather desc-gen on the Pool engine (~3.6 ns/idx) is the main cost.
  - DVE builds bf16 one-hot masks via tensor_scalar is_equal (2x rate) and
    multiplies; the d-reduction is split 2/3 DVE reduce_sum / 1/3 ACT
    accumulate; ACT applies sigmoid; the host casts and unpermutes.
"""

import numpy as np
import ml_dtypes

BF16 = ml_dtypes.bfloat16

N_DRUG, N_DIS, D = 8000, 18000, 128
N_REL_DIR, E = 3, 200000
N_CORES = 8
EPC = E // N_CORES          # 25000 edges per core per relation
T2 = 200                    # chunks per (relation, core); 25600 edge slots
EL = T2 * 128

_cache = {}
_last = {}


def _blk_of(t, nb):
    return min(t * (nb - 1) // T2, nb - 2)


def _build_nc(cfg):
    import concourse.bacc as bacc
    import concourse.mybir as mybir
    from concourse.tile import TileContext

    f32 = mybir.dt.float32
    bf16 = mybir.dt.bfloat16
    i16 = mybir.dt.int16
    u8 = mybir.dt.uint8

    nblk_f, nblk_r, _t2 = cfg
    assert _t2 == T2
    nblk = {0: nblk_f, 1: nblk_r}

    nc = bacc.Bacc("TRN2", target_bir_lowering=False, debug=False,
                   num_devices=N_CORES, num_swdge_queues=4)

    t_hd = nc.dram_tensor("hd", (N_DRUG, D), f32, kind="ExternalInput")
    t_hs = nc.dram_tensor("hs", (N_DIS, D), f32, kind="ExternalInput")
    t_u = [nc.dram_tensor(f"u{r}", (nblk[r >= 3] * 128, D), bf16,
                          kind="ExternalInput") for r in range(6)]
    t_iota = nc.dram_tensor("iota", (128, 2), f32, kind="ExternalInput")
    t_ids = [nc.dram_tensor(f"ids{r}", (128, EL), u8,
                            kind="ExternalInput") for r in range(6)]
    t_iv = [nc.dram_tensor(f"iv{r}", (128, EL // 16), i16,
                           kind="ExternalInput") for r in range(6)]
    t_out = [nc.dram_tensor(f"scores{r}", (128, T2), f32,
                            kind="ExternalOutput") for r in range(6)]

    with TileContext(nc) as tc:
        with tc.tile_pool(name="cst", bufs=1) as cst, \
             tc.tile_pool(name="mp", bufs=2) as mp, \
             tc.tile_pool(name="gp", bufs=2) as gp, \
             tc.tile_pool(name="gvp", bufs=3) as gvp, \
             tc.tile_pool(name="pp", bufs=4, space="PSUM") as pp:
            iota = cst.tile([128, 2], f32)
            nc.sync.dma_start(iota[:], t_iota[:])
            for r in range(6):
                dr = int(r >= 3)
                NB = nblk[dr]
                v_tab = t_hs if dr == 0 else t_hd

                # u range -> SBUF (row 128b+p at [p, b, :]); W-prescaled
                # bf16 on the host
                u_sb = mp.tile([128, NB, D], bf16, tag=f"usb{dr}")
                nc.sync.dma_start(
                    u_sb[:], t_u[r][:].rearrange("(b p) d -> p b d", p=128))

                iv = mp.tile([128, EL // 16], i16, tag="iv")
                nc.sync.dma_start(iv[:], t_iv[r][:])
                scores = mp.tile([128, T2], f32, tag="scores")

                batches = [40] * (T2 // 40) + ([T2 % 40] if T2 % 40 else [])
                c0 = 0
                for b, kbn in enumerate(batches):
                    nb_i = kbn * 128
                    gv = gvp.tile([128, 40, D], f32, tag="gv")
                    # split each batch across the 4 SWDGE queues: desc-gen for
                    # queue q runs on Q7 core pair q, so the four quarters
                    # generate concurrently
                    # the queue that also carries this batch's u-gather gets
                    # a smaller v share so per-pair desc-gen is balanced
                    # fine-grained, pair-balanced issue: pairs 0/1 take
                    # 2x7 v-chunks, pairs 2/3 take 6 v-chunks (they also
                    # carry the 8-chunk u-gathers) -> 14 chunks per pair
                    gx = 0
                    qn = max(1, -(-kbn // 4))
                    segs = []
                    left, q = kbn, 0
                    while left > 0:
                        take = min(qn, left)
                        segs.append((q % 4, take))
                        left -= take
                        q += 1
                    k0 = 0
                    for q, sz in segs:
                        k1 = k0 + sz
                        nc.gpsimd.dma_gather(
                            gv[:, k0:k1, :], v_tab[:],
                            iv[:, (c0 + k0) * 8:(c0 + k1) * 8],
                            sz * 128, sz * 128, D,
                            elem_step=D, single_packet=False, queue_num=q)
                        k0 = k1
                    noh = kbn
                    ids = gp.tile([128, 40 * 128], u8, tag="ids")
                    nc.sync.dma_start(
                        ids[:, :noh * 128],
                        t_ids[r][:, c0 * 128:(c0 + kbn) * 128])
                    oh_lo = gp.tile([128, 40 * 128], bf16, tag="ohlo")
                    oh_hi = gp.tile([128, 40 * 128], bf16, tag="ohhi")
                    nc.vector.tensor_scalar(
                        oh_lo[:, :noh * 128], ids[:, :noh * 128],
                        iota[:, 0:1], None, op0=mybir.AluOpType.is_equal)
                    nc.vector.tensor_scalar(
                        oh_hi[:, :noh * 128], ids[:, :noh * 128],
                        iota[:, 1:2], None, op0=mybir.AluOpType.is_equal)
                    for g0 in range(0, kbn, 4):
                        gn = min(4, kbn - g0)
                        if g0 + gn <= gx:
                            usrc = gu[:, g0:g0 + gn, :]
                        elif g0 >= gx:
                            ps = pp.tile([128, 4, D], f32, tag="ps")
                            for i in range(g0, g0 + gn):
                                t = c0 + i
                                blk = _blk_of(t, NB)
                                j = i - gx
                                nc.tensor.matmul(
                                    ps[:, i - g0, :],
                                    lhsT=oh_lo[:, j * 128:(j + 1) * 128],
                                    rhs=u_sb[:, blk, :],
                                    start=True, stop=False)
                                nc.tensor.matmul(
                                    ps[:, i - g0, :],
                                    lhsT=oh_hi[:, j * 128:(j + 1) * 128],
                                    rhs=u_sb[:, blk + 1, :],
                                    start=False, stop=True)
                            usrc = ps[:, :gn, :]
                        else:
                            raise AssertionError("gx must be multiple of 4")
                        prod = gp.tile([128, 4, D], f32, tag="prod")
                        nc.vector.tensor_tensor(
                            prod[:, :gn, :].rearrange("p a b -> p (a b)"),
                            usrc.rearrange("p a b -> p (a b)"),
                            gv[:, g0:g0 + gn, :].rearrange("p a b -> p (a b)"),
                            op=mybir.AluOpType.mult)
                        # reduction split between scalar engine (4x slower
                        # per chunk but otherwise idle) and DVE
                        if (g0 // 4) % 3 != 0:
                            nc.vector.reduce_sum(
                                scores[:, c0 + g0:c0 + g0 + gn],
                                prod[:, :gn, :], axis=mybir.AxisListType.X)
                        else:
                            acts = cst.tile([128, D], f32, tag="actout")
                            for i in range(gn):
                                nc.scalar.activation(
                                    acts[:], prod[:, i, :],
                                    mybir.ActivationFunctionType.Copy,
                                    accum_out=scores[:, c0 + g0 + i:c0 + g0 + i + 1])
                    c0 += kbn

                sig = mp.tile([128, T2], f32, tag="sig")
                nc.scalar.activation(
                    sig[:], scores[:], mybir.ActivationFunctionType.Sigmoid)
                nc.sync.dma_start(t_out[r][:], sig[:])

    nc.compile()
    return nc


def _wrap_idx(idx):
    n = idx.shape[0]
    w = idx.reshape(n // 16, 16).T.astype(np.int16)
    return np.ascontiguousarray(np.tile(w, (8, 1)))


def _pack_schedule(u_local, v_idx, nblk):
    """Greedy pack sorted edges into T2 chunks of 128 under the two-block
    window [128*B_t, 128*(B_t+2)).  Returns (ids_u8, v16, slot_of_edge)."""
    n = u_local.shape[0]
    ids = np.zeros(EL, np.uint8)
    v16 = np.zeros(EL, np.int16)
    edge_of_slot = np.full(EL, -1, np.int64)
    ptr = 0
    for t in range(T2):
        bt = min(t * (nblk - 1) // T2, nblk - 2)
        lo_row, hi_row = 128 * bt, 128 * (bt + 2)
        if ptr < n and u_local[ptr] < lo_row:
            raise RuntimeError("schedule fell behind data")
        # edges are sorted; find how many fit this window
        hi = np.searchsorted(u_local, hi_row, side="left")
        take = min(128, hi - ptr)
        if take > 0:
            s0 = t * 128
            ids[s0:s0 + take] = (u_local[ptr:ptr + take] - lo_row).astype(np.uint8)
            v16[s0:s0 + take] = v_idx[ptr:ptr + take].astype(np.int16)
            edge_of_slot[s0:s0 + take] = np.arange(ptr, ptr + take)
            # dummy slots replicate window base row with v=0 (harmless)
            ptr += take
    if ptr != n:
        raise RuntimeError(f"schedule failed to place all edges ({ptr}/{n})")
    return ids, v16, edge_of_slot


def _prepare(rels, sliced, nblk_f, nblk_r, W, iota, h_drug, h_disease):
    slot_maps = [[None] * N_CORES for _ in range(6)]
    in_maps = []
    for c in range(N_CORES):
        m = {"hd": h_drug, "hs": h_disease, "iota": iota}
        for r in range(6):
            dr = int(r >= 3)
            nblk = nblk_f if dr == 0 else nblk_r
            u_local, v_idx, lo = sliced[r][c]
            # Remap this core's rows to virtual rows spread by edge-count CDF
            # over [0, 128*(nblk-1)), so the data tracks the shared linear
            # chunk->block schedule exactly on every core.
            span = int(u_local[-1]) + 1
            V = 128 * (nblk - 1)
            counts = np.bincount(u_local, minlength=span).astype(np.int64)
            cum = np.concatenate([[0], np.cumsum(counts)[:-1]])
            target = (cum * V) // max(int(counts.sum()), 1)
            # strictly increasing: vpos[j] = max(target[j], vpos[j-1]+1)
            vpos = np.maximum.accumulate(target - np.arange(span)) + np.arange(span)
            if not vpos[-1] < nblk * 128:
                raise RuntimeError("virtual row remap overflow")
            u_virt = vpos[u_local]
            ids, v16, edge_of_slot = _pack_schedule(u_virt, v_idx, nblk)
            nrows = nblk * 128
            tab = rels[r][2]
            urows = np.zeros((nrows, D), np.float32)
            nn = min(span, tab.shape[0] - lo)
            urows[vpos[:nn]] = tab[lo:lo + nn] * W[r][None, :]
            m[f"u{r}"] = urows.astype(BF16)
            m[f"ids{r}"] = np.ascontiguousarray(
                np.broadcast_to(ids[None, :], (128, EL)))
            m[f"iv{r}"] = _wrap_idx(v16)
            slot_maps[r][c] = edge_of_slot
        in_maps.append(m)
    return slot_maps, in_maps


def kernel(h_drug, h_disease, W, drug_src, dis_dst, dis_src, drug_dst):
    from concourse.bass_utils import run_bass_kernel_spmd

    h_drug = np.asarray(h_drug, dtype=np.float32)
    h_disease = np.asarray(h_disease, dtype=np.float32)
    W = np.asarray(W, dtype=np.float32)

    rels = []
    for r in range(3):
        rels.append((np.asarray(drug_src[r]), np.asarray(dis_dst[r]), h_drug))
    for r in range(3):
        rels.append((np.asarray(dis_src[r]), np.asarray(drug_dst[r]), h_disease))

    perms = []
    sliced = []
    for r in range(6):
        u_idx, v_idx, _ = rels[r]
        perm = np.argsort(u_idx, kind="stable")
        perms.append(perm)
        us, vs = u_idx[perm], v_idx[perm]
        sl = []
        for c in range(N_CORES):
            ui = us[c * EPC:(c + 1) * EPC]
            vi = vs[c * EPC:(c + 1) * EPC]
            lo = int(ui[0])
            sl.append((ui - lo, vi, lo))
        sliced.append(sl)

    def max_blocks(dr):
        nb = 2
        for r in (range(3) if dr == 0 else range(3, 6)):
            for c in range(N_CORES):
                u_local = sliced[r][c][0]
                nb = max(nb, int(u_local[-1]) // 128 + 2)
        return nb

    nblk_f, nblk_r = max_blocks(0), max_blocks(1)

    iota = np.empty((128, 2), np.float32)
    iota[:, 0] = np.arange(128)
    iota[:, 1] = np.arange(128, 256)

    global T2, EL
    for _attempt in range(4):
        try:
            slot_maps, in_maps = _prepare(rels, sliced, nblk_f, nblk_r,
                                          W, iota, h_drug, h_disease)
            break
        except RuntimeError:
            # pathological row distribution: give the schedule more slack
            T2 += 8
            EL = T2 * 128
    else:
        raise RuntimeError("could not build a feasible chunk schedule")

    cfg = (nblk_f, nblk_r, T2)
    if cfg not in _cache:
        _cache[cfg] = _build_nc(cfg)
    nc = _cache[cfg]

    res = run_bass_kernel_spmd(nc, in_maps, core_ids=list(range(N_CORES)))
    _last["exec_time_ns"] = res.exec_time_ns
    if res.instructions_and_trace is not None:
        _last["trace_path"] = res.instructions_and_trace[1]

    out = np.empty((6, E), np.float32)
    for r in range(6):
        sorted_scores = np.empty(EPC * N_CORES, np.float32)
        for c in range(N_CORES):
            s = res.results[c][f"scores{r}"]       # [128, T2]
            flat = s.T.reshape(-1)                 # slot j = t*128+p
            eos = slot_maps[r][c]
            valid = eos >= 0
            sorted_scores[c * EPC + eos[valid]] = flat[valid]
        out[r, perms[r]] = sorted_scores
    return out



# revision 18
# speedup vs baseline: 2.3323x; 1.0173x over previous
"""DistMult edge scorer on 8 Trainium2 NeuronCores.

score[r, e] = sigmoid(sum_d h_u[src[r,e], d] * W[r, d] * h_v[dst[r,e], d])

Sharding: edges of each relation are sorted by source node on the host and
split into 8 contiguous slices (one per core).

Per core, per relation:
  - u side: the core's contiguous source-row range is W-prescaled on the
    host, cast to bf16, DMA'd into SBUF, and expanded per edge by bf16 PE
    one-hot matmuls (no u gathers).  Chunk t of 128 edges may only use
    source rows inside a two-block window [128*B_t, 128*(B_t+2)) with
    B_t = floor(t*NBLK/T2) fixed at compile time; the host packs edges
    greedily into chunks under that constraint.
  - v side: per-edge f32 rows fetched with SWDGE dma_gather (512B rows,
    edges-on-partitions), 4 sem-congruent calls per 40-chunk batch.  The
    gather desc-gen on the Pool engine (~3.6 ns/idx) is the main cost.
  - DVE builds bf16 one-hot masks via tensor_scalar is_equal (2x rate) and
    multiplies; the d-reduction is split 2/3 DVE reduce_sum / 1/3 ACT
    accumulate; ACT applies sigmoid; the host casts and unpermutes.
"""

import numpy as np
import ml_dtypes

BF16 = ml_dtypes.bfloat16

N_DRUG, N_DIS, D = 8000, 18000, 128
N_REL_DIR, E = 3, 200000
N_CORES = 8
EPC = E // N_CORES          # 25000 edges per core per relation
T2 = 208                    # chunks per (relation, core); 26624 edge slots
EL = T2 * 128

_cache = {}
_last = {}


def _blk_of(t, nb):
    return t * nb // T2


def _build_nc(cfg):
    import concourse.bacc as bacc
    import concourse.mybir as mybir
    from concourse.tile import TileContext

    f32 = mybir.dt.float32
    bf16 = mybir.dt.bfloat16
    i16 = mybir.dt.int16
    u8 = mybir.dt.uint8

    nblk_f, nblk_r, _t2 = cfg
    assert _t2 == T2
    nblk = {0: nblk_f, 1: nblk_r}

    nc = bacc.Bacc("TRN2", target_bir_lowering=False, debug=False,
                   num_devices=N_CORES, num_swdge_queues=4)

    t_hd = nc.dram_tensor("hd", (N_DRUG, D), f32, kind="ExternalInput")
    t_hs = nc.dram_tensor("hs", (N_DIS, D), f32, kind="ExternalInput")
    t_u = [nc.dram_tensor(f"u{r}", (nblk[r >= 3] * 128, D), bf16,
                          kind="ExternalInput") for r in range(6)]
    t_iota = nc.dram_tensor("iota", (128, 2), f32, kind="ExternalInput")
    t_ids = [nc.dram_tensor(f"ids{r}", (128, EL), u8,
                            kind="ExternalInput") for r in range(6)]
    t_iv = [nc.dram_tensor(f"iv{r}", (128, EL // 16), i16,
                           kind="ExternalInput") for r in range(6)]
    t_out = [nc.dram_tensor(f"scores{r}", (128, T2), f32,
                            kind="ExternalOutput") for r in range(6)]

    with TileContext(nc) as tc:
        with tc.tile_pool(name="cst", bufs=1) as cst, \
             tc.tile_pool(name="mp", bufs=2) as mp, \
             tc.tile_pool(name="gp", bufs=2) as gp, \
             tc.tile_pool(name="gvp", bufs=3) as gvp, \
             tc.tile_pool(name="pp", bufs=4, space="PSUM") as pp:
            iota = cst.tile([128, 2], f32)
            nc.sync.dma_start(iota[:], t_iota[:])
            for r in range(6):
                dr = int(r >= 3)
                NB = nblk[dr]
                v_tab = t_hs if dr == 0 else t_hd

                # u range -> SBUF (row 128b+p at [p, b, :]); W-prescaled
                # bf16 on the host
                u_sb = mp.tile([128, NB, D], bf16, tag=f"usb{dr}")
                nc.sync.dma_start(
                    u_sb[:], t_u[r][:].rearrange("(b p) d -> p b d", p=128))

                iv = mp.tile([128, EL // 16], i16, tag="iv")
                nc.sync.dma_start(iv[:], t_iv[r][:])
                scores = mp.tile([128, T2], f32, tag="scores")

                batches = [40] * (T2 // 40) + ([T2 % 40] if T2 % 40 else [])
                c0 = 0
                for b, kbn in enumerate(batches):
                    nb_i = kbn * 128
                    gv = gvp.tile([128, 40, D], f32, tag="gv")
                    # split each batch across the 4 SWDGE queues: desc-gen for
                    # queue q runs on Q7 core pair q, so the four quarters
                    # generate concurrently
                    # the queue that also carries this batch's u-gather gets
                    # a smaller v share so per-pair desc-gen is balanced
                    # fine-grained, pair-balanced issue: pairs 0/1 take
                    # 2x7 v-chunks, pairs 2/3 take 6 v-chunks (they also
                    # carry the 8-chunk u-gathers) -> 14 chunks per pair
                    gx = 0
                    qn = max(1, -(-kbn // 4))
                    segs = []
                    left, q = kbn, 0
                    while left > 0:
                        take = min(qn, left)
                        segs.append((q % 4, take))
                        left -= take
                        q += 1
                    k0 = 0
                    for q, sz in segs:
                        k1 = k0 + sz
                        nc.gpsimd.dma_gather(
                            gv[:, k0:k1, :], v_tab[:],
                            iv[:, (c0 + k0) * 8:(c0 + k1) * 8],
                            sz * 128, sz * 128, D,
                            elem_step=D, single_packet=False, queue_num=q)
                        k0 = k1
                    noh = kbn
                    ids = gp.tile([128, 40 * 128], u8, tag="ids")
                    nc.sync.dma_start(
                        ids[:, :noh * 128],
                        t_ids[r][:, c0 * 128:(c0 + kbn) * 128])
                    oh_lo = gp.tile([128, 40 * 128], bf16, tag="ohlo")
                    nc.vector.tensor_scalar(
                        oh_lo[:, :noh * 128], ids[:, :noh * 128],
                        iota[:, 1:2], None, op0=mybir.AluOpType.is_equal)
                    for g0 in range(0, kbn, 4):
                        gn = min(4, kbn - g0)
                        if g0 + gn <= gx:
                            usrc = gu[:, g0:g0 + gn, :]
                        elif g0 >= gx:
                            ps = pp.tile([128, 4, D], f32, tag="ps")
                            for i in range(g0, g0 + gn):
                                t = c0 + i
                                blk = _blk_of(t, NB)
                                j = i - gx
                                nc.tensor.matmul(
                                    ps[:, i - g0, :],
                                    lhsT=oh_lo[:, j * 128:(j + 1) * 128],
                                    rhs=u_sb[:, blk, :],
                                    start=True, stop=True)
                            usrc = ps[:, :gn, :]
                        else:
                            raise AssertionError("gx must be multiple of 4")
                        prod = gp.tile([128, 4, D], f32, tag="prod")
                        nc.vector.tensor_tensor(
                            prod[:, :gn, :].rearrange("p a b -> p (a b)"),
                            usrc.rearrange("p a b -> p (a b)"),
                            gv[:, g0:g0 + gn, :].rearrange("p a b -> p (a b)"),
                            op=mybir.AluOpType.mult)
                        # reduction split between scalar engine (4x slower
                        # per chunk but otherwise idle) and DVE
                        if (g0 // 4) % 3 != 0:
                            nc.vector.reduce_sum(
                                scores[:, c0 + g0:c0 + g0 + gn],
                                prod[:, :gn, :], axis=mybir.AxisListType.X)
                        else:
                            acts = cst.tile([128, D], f32, tag="actout")
                            for i in range(gn):
                                nc.scalar.activation(
                                    acts[:], prod[:, i, :],
                                    mybir.ActivationFunctionType.Copy,
                                    accum_out=scores[:, c0 + g0 + i:c0 + g0 + i + 1])
                    c0 += kbn

                sig = mp.tile([128, T2], f32, tag="sig")
                nc.scalar.activation(
                    sig[:], scores[:], mybir.ActivationFunctionType.Sigmoid)
                nc.sync.dma_start(t_out[r][:], sig[:])

    nc.compile()
    return nc


def _wrap_idx(idx):
    n = idx.shape[0]
    w = idx.reshape(n // 16, 16).T.astype(np.int16)
    return np.ascontiguousarray(np.tile(w, (8, 1)))


def _pack_schedule(u_local, v_idx, nblk):
    """Pack edges (sorted by u_local) into T2 chunks of 128 where chunk t may
    only use rows assigned to block blk_t = t*nblk//T2, at most 128 distinct
    rows per block.  Returns (ids, v16, edge_of_slot, vpos) or None."""
    rows, starts, counts = np.unique(u_local, return_index=True,
                                     return_counts=True)
    nrows = rows.shape[0]
    ids = np.zeros(EL, np.uint8)
    v16 = np.zeros(EL, np.int16)
    eos = np.full(EL, -1, np.int64)
    vpos = np.full(int(u_local[-1]) + 1, -1, np.int64)

    blk_of_t = np.array([_blk_of(t, nblk) for t in range(T2)], np.int64)
    t_first = np.searchsorted(blk_of_t, np.arange(nblk), side="left")
    t_last = np.searchsorted(blk_of_t, np.arange(nblk), side="right")
    ri = 0
    for b in range(nblk):
        cap = 128 * int(t_last[b] - t_first[b])
        slot0 = 128 * int(t_first[b])
        used = 0
        rib = 0
        while ri < nrows and rib < 128 and used + int(counts[ri]) <= cap:
            c = int(counts[ri])
            sx = int(starts[ri])
            sl = slot0 + used
            ids[sl:sl + c] = rib
            v16[sl:sl + c] = v_idx[sx:sx + c].astype(np.int16)
            eos[sl:sl + c] = np.arange(sx, sx + c)
            vpos[int(rows[ri])] = 128 * b + rib
            used += c
            rib += 1
            ri += 1
    if ri != nrows:
        return None
    return ids, v16, eos, vpos


def _prepare(rels, sliced, nblk_f, nblk_r, W, iota):
    slot_maps = [[None] * N_CORES for _ in range(6)]
    in_maps = []
    for c in range(N_CORES):
        m = {"hd": rels[0][2], "hs": rels[3][2], "iota": iota}
        for r in range(6):
            dr = int(r >= 3)
            nblk = nblk_f if dr == 0 else nblk_r
            u_local, v_idx, lo = sliced[r][c]
            packed = _pack_schedule(u_local, v_idx, nblk)
            if packed is None:
                return None, None, (r, c)
            ids, v16, eos, vpos = packed
            tab = rels[r][2]
            span = vpos.shape[0]
            urows = np.zeros((nblk * 128, D), np.float32)
            valid = vpos >= 0
            urows[vpos[valid]] = (tab[lo:lo + span][valid]
                                  * W[r][None, :]).astype(np.float32)
            m[f"u{r}"] = urows.astype(BF16)
            m[f"ids{r}"] = np.ascontiguousarray(
                np.broadcast_to(ids[None, :], (128, EL)))
            m[f"iv{r}"] = _wrap_idx(v16)
            slot_maps[r][c] = eos
        in_maps.append(m)
    return slot_maps, in_maps, None


def kernel(h_drug, h_disease, W, drug_src, dis_dst, dis_src, drug_dst):
    from concourse.bass_utils import run_bass_kernel_spmd

    h_drug = np.asarray(h_drug, dtype=np.float32)
    h_disease = np.asarray(h_disease, dtype=np.float32)
    W = np.asarray(W, dtype=np.float32)

    rels = []
    for r in range(3):
        rels.append((np.asarray(drug_src[r]), np.asarray(dis_dst[r]), h_drug))
    for r in range(3):
        rels.append((np.asarray(dis_src[r]), np.asarray(drug_dst[r]), h_disease))

    perms = []
    sliced = []
    for r in range(6):
        u_idx, v_idx, _ = rels[r]
        perm = np.argsort(u_idx, kind="stable")
        perms.append(perm)
        us, vs = u_idx[perm], v_idx[perm]
        sl = []
        for c in range(N_CORES):
            ui = us[c * EPC:(c + 1) * EPC]
            vi = vs[c * EPC:(c + 1) * EPC]
            lo = int(ui[0])
            sl.append((ui - lo, vi, lo))
        sliced.append(sl)

    def span_max(dr):
        sp = 0
        for r in (range(3) if dr == 0 else range(3, 6)):
            for c in range(N_CORES):
                sp = max(sp, int(sliced[r][c][0][-1]) + 1)
        return sp

    nblk_f = max(2, -(-span_max(0) // 112))
    nblk_r = max(2, -(-span_max(1) // 112))

    iota = np.empty((128, 2), np.float32)
    iota[:, 0] = np.arange(128)
    iota[:, 1] = np.arange(128)

    slot_maps = in_maps = None
    for _attempt in range(6):
        slot_maps, in_maps, fail = _prepare(rels, sliced, nblk_f, nblk_r,
                                            W, iota)
        if fail is None:
            break
        if fail[0] < 3:
            nblk_f += 1
        else:
            nblk_r += 1
    else:
        raise RuntimeError("could not build a feasible chunk schedule")

    cfg = (nblk_f, nblk_r, T2)
    if cfg not in _cache:
        _cache[cfg] = _build_nc(cfg)
    nc = _cache[cfg]

    res = run_bass_kernel_spmd(nc, in_maps, core_ids=list(range(N_CORES)))
    _last["exec_time_ns"] = res.exec_time_ns
    if res.instructions_and_trace is not None:
        _last["trace_path"] = res.instructions_and_trace[1]

    out = np.empty((6, E), np.float32)
    for r in range(6):
        sorted_scores = np.empty(EPC * N_CORES, np.float32)
        for c in range(N_CORES):
            s = res.results[c][f"scores{r}"]       # [128, T2]
            flat = s.T.reshape(-1)                 # slot j = t*128+p
            eos = slot_maps[r][c]
            valid = eos >= 0
            sorted_scores[c * EPC + eos[valid]] = flat[valid]
        out[r, perms[r]] = sorted_scores
    return out



# revision 19
# speedup vs baseline: 2.4314x; 1.0425x over previous
"""DistMult edge scorer on 8 Trainium2 NeuronCores.

score[r, e] = sigmoid(sum_d h_u[src[r,e], d] * W[r, d] * h_v[dst[r,e], d])

Sharding: edges of each relation are sorted by source node on the host and
split into 8 contiguous slices (one per core).

Per core, per relation:
  - u side: the core's contiguous source-row range is W-prescaled on the
    host, cast to bf16, DMA'd into SBUF, and expanded per edge by bf16 PE
    one-hot matmuls (no u gathers).  Chunk t of 128 edges may only use
    source rows inside a two-block window [128*B_t, 128*(B_t+2)) with
    B_t = floor(t*NBLK/T2) fixed at compile time; the host packs edges
    greedily into chunks under that constraint.
  - v side: per-edge f32 rows fetched with SWDGE dma_gather (512B rows,
    edges-on-partitions), 4 sem-congruent calls per 40-chunk batch.  The
    gather desc-gen on the Pool engine (~3.6 ns/idx) is the main cost.
  - DVE builds bf16 one-hot masks via tensor_scalar is_equal (2x rate) and
    multiplies; the d-reduction is split 2/3 DVE reduce_sum / 1/3 ACT
    accumulate; ACT applies sigmoid; the host casts and unpermutes.
"""

import numpy as np
import ml_dtypes

BF16 = ml_dtypes.bfloat16

N_DRUG, N_DIS, D = 8000, 18000, 128
N_REL_DIR, E = 3, 200000
N_CORES = 8
EPC = E // N_CORES          # 25000 edges per core per relation
T2 = 208                    # chunks per (relation, core); 26624 edge slots
EL = T2 * 128

_cache = {}
_last = {}


def _blk_of(t, nb):
    return t * nb // T2


def _build_nc(cfg):
    import concourse.bacc as bacc
    import concourse.mybir as mybir
    from concourse.tile import TileContext

    f32 = mybir.dt.float32
    bf16 = mybir.dt.bfloat16
    i16 = mybir.dt.int16
    u8 = mybir.dt.uint8

    nblk_f, nblk_r, _t2 = cfg
    assert _t2 == T2
    nblk = {0: nblk_f, 1: nblk_r}

    nc = bacc.Bacc("TRN2", target_bir_lowering=False, debug=False,
                   num_devices=N_CORES, num_swdge_queues=4)

    t_hd = nc.dram_tensor("hd", (N_DRUG, D), bf16, kind="ExternalInput")
    t_hs = nc.dram_tensor("hs", (N_DIS, D), bf16, kind="ExternalInput")
    t_u = [nc.dram_tensor(f"u{r}", (nblk[r >= 3] * 128, D), bf16,
                          kind="ExternalInput") for r in range(6)]
    t_iota = nc.dram_tensor("iota", (128, 2), f32, kind="ExternalInput")
    t_ids = [nc.dram_tensor(f"ids{r}", (128, EL), u8,
                            kind="ExternalInput") for r in range(6)]
    t_iv = [nc.dram_tensor(f"iv{r}", (128, EL // 16), i16,
                           kind="ExternalInput") for r in range(6)]
    t_out = [nc.dram_tensor(f"scores{r}", (128, T2), f32,
                            kind="ExternalOutput") for r in range(6)]

    with TileContext(nc) as tc:
        with tc.tile_pool(name="cst", bufs=1) as cst, \
             tc.tile_pool(name="mp", bufs=2) as mp, \
             tc.tile_pool(name="gp", bufs=2) as gp, \
             tc.tile_pool(name="gvp", bufs=3) as gvp, \
             tc.tile_pool(name="pp", bufs=4, space="PSUM") as pp:
            iota = cst.tile([128, 2], f32)
            nc.sync.dma_start(iota[:], t_iota[:])
            for r in range(6):
                dr = int(r >= 3)
                NB = nblk[dr]
                v_tab = t_hs if dr == 0 else t_hd

                # u range -> SBUF (row 128b+p at [p, b, :]); W-prescaled
                # bf16 on the host
                u_sb = mp.tile([128, NB, D], bf16, tag=f"usb{dr}")
                nc.sync.dma_start(
                    u_sb[:], t_u[r][:].rearrange("(b p) d -> p b d", p=128))

                iv = mp.tile([128, EL // 16], i16, tag="iv")
                nc.sync.dma_start(iv[:], t_iv[r][:])
                scores = mp.tile([128, T2], f32, tag="scores")

                batches = [40] * (T2 // 40) + ([T2 % 40] if T2 % 40 else [])
                c0 = 0
                for b, kbn in enumerate(batches):
                    nb_i = kbn * 128
                    gv = gvp.tile([128, 40, D], bf16, tag="gv")
                    # split each batch across the 4 SWDGE queues: desc-gen for
                    # queue q runs on Q7 core pair q, so the four quarters
                    # generate concurrently
                    # the queue that also carries this batch's u-gather gets
                    # a smaller v share so per-pair desc-gen is balanced
                    # fine-grained, pair-balanced issue: pairs 0/1 take
                    # 2x7 v-chunks, pairs 2/3 take 6 v-chunks (they also
                    # carry the 8-chunk u-gathers) -> 14 chunks per pair
                    gx = 0
                    qn = max(1, -(-kbn // 4))
                    segs = []
                    left, q = kbn, 0
                    while left > 0:
                        take = min(qn, left)
                        segs.append((q % 4, take))
                        left -= take
                        q += 1
                    k0 = 0
                    for q, sz in segs:
                        k1 = k0 + sz
                        nc.gpsimd.dma_gather(
                            gv[:, k0:k1, :], v_tab[:],
                            iv[:, (c0 + k0) * 8:(c0 + k1) * 8],
                            sz * 128, sz * 128, D,
                            elem_step=D, single_packet=False, queue_num=q)
                        k0 = k1
                    noh = kbn
                    ids = gp.tile([128, 40 * 128], u8, tag="ids")
                    nc.sync.dma_start(
                        ids[:, :noh * 128],
                        t_ids[r][:, c0 * 128:(c0 + kbn) * 128])
                    oh_lo = gp.tile([128, 40 * 128], bf16, tag="ohlo")
                    nc.vector.tensor_scalar(
                        oh_lo[:, :noh * 128], ids[:, :noh * 128],
                        iota[:, 1:2], None, op0=mybir.AluOpType.is_equal)
                    for g0 in range(0, kbn, 4):
                        gn = min(4, kbn - g0)
                        if g0 + gn <= gx:
                            usrc = gu[:, g0:g0 + gn, :]
                        elif g0 >= gx:
                            ps = pp.tile([128, 4, D], f32, tag="ps")
                            for i in range(g0, g0 + gn):
                                t = c0 + i
                                blk = _blk_of(t, NB)
                                j = i - gx
                                nc.tensor.matmul(
                                    ps[:, i - g0, :],
                                    lhsT=oh_lo[:, j * 128:(j + 1) * 128],
                                    rhs=u_sb[:, blk, :],
                                    start=True, stop=True)
                            usrc = ps[:, :gn, :]
                        else:
                            raise AssertionError("gx must be multiple of 4")
                        prod = gp.tile([128, 4, D], f32, tag="prod")
                        nc.vector.tensor_tensor(
                            prod[:, :gn, :].rearrange("p a b -> p (a b)"),
                            usrc.rearrange("p a b -> p (a b)"),
                            gv[:, g0:g0 + gn, :].rearrange("p a b -> p (a b)"),
                            op=mybir.AluOpType.mult)
                        # reduction split between scalar engine (4x slower
                        # per chunk but otherwise idle) and DVE
                        if (g0 // 4) % 3 != 0:
                            nc.vector.reduce_sum(
                                scores[:, c0 + g0:c0 + g0 + gn],
                                prod[:, :gn, :], axis=mybir.AxisListType.X)
                        else:
                            acts = cst.tile([128, D], f32, tag="actout")
                            for i in range(gn):
                                nc.scalar.activation(
                                    acts[:], prod[:, i, :],
                                    mybir.ActivationFunctionType.Copy,
                                    accum_out=scores[:, c0 + g0 + i:c0 + g0 + i + 1])
                    c0 += kbn

                sig = mp.tile([128, T2], f32, tag="sig")
                nc.scalar.activation(
                    sig[:], scores[:], mybir.ActivationFunctionType.Sigmoid)
                nc.sync.dma_start(t_out[r][:], sig[:])

    nc.compile()
    return nc


def _wrap_idx(idx):
    n = idx.shape[0]
    w = idx.reshape(n // 16, 16).T.astype(np.int16)
    return np.ascontiguousarray(np.tile(w, (8, 1)))


def _pack_schedule(u_local, v_idx, nblk):
    """Pack edges (sorted by u_local) into T2 chunks of 128 where chunk t may
    only use rows assigned to block blk_t = t*nblk//T2, at most 128 distinct
    rows per block.  Returns (ids, v16, edge_of_slot, vpos) or None."""
    rows, starts, counts = np.unique(u_local, return_index=True,
                                     return_counts=True)
    nrows = rows.shape[0]
    ids = np.zeros(EL, np.uint8)
    v16 = np.zeros(EL, np.int16)
    eos = np.full(EL, -1, np.int64)
    vpos = np.full(int(u_local[-1]) + 1, -1, np.int64)

    blk_of_t = np.array([_blk_of(t, nblk) for t in range(T2)], np.int64)
    t_first = np.searchsorted(blk_of_t, np.arange(nblk), side="left")
    t_last = np.searchsorted(blk_of_t, np.arange(nblk), side="right")
    ri = 0
    for b in range(nblk):
        cap = 128 * int(t_last[b] - t_first[b])
        slot0 = 128 * int(t_first[b])
        used = 0
        rib = 0
        while ri < nrows and rib < 128 and used + int(counts[ri]) <= cap:
            c = int(counts[ri])
            sx = int(starts[ri])
            sl = slot0 + used
            ids[sl:sl + c] = rib
            v16[sl:sl + c] = v_idx[sx:sx + c].astype(np.int16)
            eos[sl:sl + c] = np.arange(sx, sx + c)
            vpos[int(rows[ri])] = 128 * b + rib
            used += c
            rib += 1
            ri += 1
    if ri != nrows:
        return None
    return ids, v16, eos, vpos


def _prepare(rels, sliced, nblk_f, nblk_r, W, iota):
    slot_maps = [[None] * N_CORES for _ in range(6)]
    in_maps = []
    hd16 = rels[0][2].astype(BF16)
    hs16 = rels[3][2].astype(BF16)
    for c in range(N_CORES):
        m = {"hd": hd16, "hs": hs16, "iota": iota}
        for r in range(6):
            dr = int(r >= 3)
            nblk = nblk_f if dr == 0 else nblk_r
            u_local, v_idx, lo = sliced[r][c]
            packed = _pack_schedule(u_local, v_idx, nblk)
            if packed is None:
                return None, None, (r, c)
            ids, v16, eos, vpos = packed
            tab = rels[r][2]
            span = vpos.shape[0]
            urows = np.zeros((nblk * 128, D), np.float32)
            valid = vpos >= 0
            urows[vpos[valid]] = (tab[lo:lo + span][valid]
                                  * W[r][None, :]).astype(np.float32)
            m[f"u{r}"] = urows.astype(BF16)
            m[f"ids{r}"] = np.ascontiguousarray(
                np.broadcast_to(ids[None, :], (128, EL)))
            m[f"iv{r}"] = _wrap_idx(v16)
            slot_maps[r][c] = eos
        in_maps.append(m)
    return slot_maps, in_maps, None


def kernel(h_drug, h_disease, W, drug_src, dis_dst, dis_src, drug_dst):
    from concourse.bass_utils import run_bass_kernel_spmd

    h_drug = np.asarray(h_drug, dtype=np.float32)
    h_disease = np.asarray(h_disease, dtype=np.float32)
    W = np.asarray(W, dtype=np.float32)

    rels = []
    for r in range(3):
        rels.append((np.asarray(drug_src[r]), np.asarray(dis_dst[r]), h_drug))
    for r in range(3):
        rels.append((np.asarray(dis_src[r]), np.asarray(drug_dst[r]), h_disease))

    perms = []
    sliced = []
    for r in range(6):
        u_idx, v_idx, _ = rels[r]
        perm = np.argsort(u_idx, kind="stable")
        perms.append(perm)
        us, vs = u_idx[perm], v_idx[perm]
        sl = []
        for c in range(N_CORES):
            ui = us[c * EPC:(c + 1) * EPC]
            vi = vs[c * EPC:(c + 1) * EPC]
            lo = int(ui[0])
            sl.append((ui - lo, vi, lo))
        sliced.append(sl)

    def span_max(dr):
        sp = 0
        for r in (range(3) if dr == 0 else range(3, 6)):
            for c in range(N_CORES):
                sp = max(sp, int(sliced[r][c][0][-1]) + 1)
        return sp

    nblk_f = max(2, -(-span_max(0) // 112))
    nblk_r = max(2, -(-span_max(1) // 112))

    iota = np.empty((128, 2), np.float32)
    iota[:, 0] = np.arange(128)
    iota[:, 1] = np.arange(128)

    slot_maps = in_maps = None
    for _attempt in range(6):
        slot_maps, in_maps, fail = _prepare(rels, sliced, nblk_f, nblk_r,
                                            W, iota)
        if fail is None:
            break
        if fail[0] < 3:
            nblk_f += 1
        else:
            nblk_r += 1
    else:
        raise RuntimeError("could not build a feasible chunk schedule")

    cfg = (nblk_f, nblk_r, T2)
    if cfg not in _cache:
        _cache[cfg] = _build_nc(cfg)
    nc = _cache[cfg]

    res = run_bass_kernel_spmd(nc, in_maps, core_ids=list(range(N_CORES)))
    _last["exec_time_ns"] = res.exec_time_ns
    if res.instructions_and_trace is not None:
        _last["trace_path"] = res.instructions_and_trace[1]

    out = np.empty((6, E), np.float32)
    for r in range(6):
        sorted_scores = np.empty(EPC * N_CORES, np.float32)
        for c in range(N_CORES):
            s = res.results[c][f"scores{r}"]       # [128, T2]
            flat = s.T.reshape(-1)                 # slot j = t*128+p
            eos = slot_maps[r][c]
            valid = eos >= 0
            sorted_scores[c * EPC + eos[valid]] = flat[valid]
        out[r, perms[r]] = sorted_scores
    return out

